# revision 12
# baseline (speedup 1.0000x reference)
"""3-layer GCN (GCNConv x3, PyG defaults) on 8 Trainium2 NeuronCores.

Strategy (graph/data parallel, per sharding hint):
  - Nodes are sharded 8 ways by destination range (6250 nodes/core, padded to
    6272-token sections of 128-feat bf16 tokens, 256 B each).
  - Per layer, one AllGather collective publishes every core's freshly
    computed p-rows (p = dinv * h) to a shared DRAM buffer `ago`
    [C*128, nranks*128]; token (core c, pos) sits at row c*128 + pos%128,
    byte offset (pos//128)*256 -- i.e. 256 B contiguous in DRAM.
  - A = D^-1/2 (A+I) D^-1/2 aggregation: per-edge gathers run on the GPSIMD
    dma_gather DIRECTLY from `ago` DRAM (no SBUF staging copy of the full
    token space); the segment-sum runs on the TensorEngine as identity-matmul
    accumulation into PSUM, slot-major with nodes sorted by degree descending
    (slot k covers the prefix of nodes with degree > k).
  - dma_gather indices are int16, so sources split into two passes by source
    core (cores 0-3 / 4-7, 25088 tokens each, re-based in_ap); the hi pass
    drains (transposed) into a token-major tile TS, which a small on-chip
    gather permutes into the lo pass's node order; the permuted tile and the
    self-loop term are merged into the lo pass's PSUM accumulation as extra
    identity-matmul slots, and the drain applies relu directly.
  - Self-loops never go through the gather: the dense-stage PSUM drain emits
    a second copy scaled by dinv^2 into an SBUF tile SL used as the PSUM
    seed slot.
  - Dense stages (X@W, act@W) run on the TensorEngine; deg^-1/2 scales fold
    into per-partition scalars of the PSUM-drain activation op.
All 8 cores run one identical program; only input data differs per core.
"""
import sys
import os

sys.path.insert(0, "/opt/trn_rl_repo")

import numpy as np
import ml_dtypes

from concourse import bass, bacc, mybir
from concourse import tile
from concourse.bass_utils import run_bass_kernel_spmd

BF16 = ml_dtypes.bfloat16
C = 8
BLK = 512
CHUNK = 8192
FOUT_PAD = 128  # W_out columns padded so layer-3 tokens share the 256B layout


# --------------------------------------------------------------------------
# Host-side preprocessing: pure integer/index work + normalization constants.
# --------------------------------------------------------------------------
class Plan:
    pass


def _pack_idx(vals):
    """int16 list -> [128, len/16] wrapped (i -> [i%16, i//16]) replicated x8."""
    n = len(vals)
    assert n % 16 == 0
    a = np.asarray(vals, dtype=np.int16).reshape(n // 16, 16).T  # [16, n/16]
    return np.tile(a, (8, 1))


def _pass_structure(jpos, toks, npc, dummy_tok, need_cover):
    """Slot-major structure for one (core, pass).

    jpos: position (by this pass's degree-desc order) of each edge's dst.
    toks: gather token id of each edge's src.
    need_cover: if True, positions with zero degree get one dummy edge so
    every position is written by some slot (required when the PSUM block has
    no seed slot).
    Returns (cells, data) where cells[(b, k)] = count and data[(b, k)] = token
    array (dst positions ascending within each cell).
    """
    cnt = np.bincount(jpos, minlength=npc)
    if need_cover:
        zpos = np.nonzero(cnt == 0)[0]
        if len(zpos):
            jpos = np.concatenate([jpos, zpos])
            toks = np.concatenate([toks, np.full(len(zpos), dummy_tok, np.int64)])
            cnt[zpos] = 1
    order = np.argsort(jpos, kind="stable")
    js = jpos[order]
    ts = toks[order]
    starts = np.zeros(npc, np.int64)
    starts[1:] = np.cumsum(cnt)[:-1]
    kk = np.arange(len(js)) - starts[js]
    bb = js // BLK
    o2 = np.lexsort((js, kk, bb))
    js, ts, kk, bb = js[o2], ts[o2], kk[o2], bb[o2]
    cells = {}
    data = {}
    cell_id = bb * 4096 + kk
    uniq, first = np.unique(cell_id, return_index=True)
    bounds = list(first) + [len(cell_id)]
    for i, u in enumerate(uniq):
        b, k = int(u) // 4096, int(u) % 4096
        seg = slice(bounds[i], bounds[i + 1])
        cells[(b, k)] = bounds[i + 1] - bounds[i]
        data[(b, k)] = ts[seg]
    return cells, data


def _chunk_cells(cells_max, nblk):
    """Pack (b,k) cells into gather chunks of <= CHUNK columns.

    Returns (chunks, cell_loc): chunks = list of padded lengths;
    cell_loc[(b,k)] = (chunk_idx, offset, n).
    """
    chunks = []
    cell_loc = {}
    cur = 0
    for b in range(nblk):
        ks = sorted(k for (bb, k) in cells_max if bb == b)
        for k in ks:
            n = cells_max[(b, k)]
            cap = 2048 if len(chunks) <= 1 else CHUNK
            if cur == 0 or chunks[-1] + n > cap:
                chunks.append(0)
                cur = 1
            cell_loc[(b, k)] = (len(chunks) - 1, chunks[-1], n)
            chunks[-1] += n
    chunks = [(l + 127) // 128 * 128 for l in chunks]
    return chunks, cell_loc


def preprocess(x, edge_index, n_nodes):
    p = Plan()
    N = n_nodes
    assert N % C == 0
    npc = N // C
    sec = (npc + 127) // 128 * 128
    p.npc, p.sec = npc, sec
    p.nranks = sec // 128
    p.nblk = (npc + BLK - 1) // BLK
    p.fin = x.shape[1]
    assert p.fin % 128 == 0
    p.finc = p.fin // 128

    src = np.asarray(edge_index[0], dtype=np.int64)
    dst = np.asarray(edge_index[1], dtype=np.int64)
    deg = (np.bincount(dst, minlength=N) + 1).astype(np.float32)
    dinv = (1.0 / np.sqrt(deg)).astype(np.float32)

    # NO self-loops in the edge lists: the dense stage emits the dinv^2-scaled
    # self term directly (SL tile).
    s_core = src // npc
    d_core = dst // npc
    lo_cut = C // 2

    # per-core degree split and orders
    perm_lo = np.empty((C, npc), np.int64)
    perm_hi = np.empty((C, npc), np.int64)
    pos_lo = np.empty(N, np.int64)
    pos_hi_local = np.empty((C, npc), np.int64)
    for m in range(C):
        sel = d_core == m
        dl = dst[sel] - m * npc
        lo = s_core[sel] < lo_cut
        cl = np.bincount(dl[lo], minlength=npc)
        ch = np.bincount(dl[~lo], minlength=npc)
        pl = np.argsort(-np.maximum(cl, 1), kind="stable")
        ph = np.argsort(-np.maximum(ch, 1), kind="stable")
        perm_lo[m], perm_hi[m] = pl, ph
        pos_lo[m * npc + pl] = np.arange(npc)
        pos_hi_local[m, ph] = np.arange(npc)
    p.perm_lo = perm_lo

    # DRAM token id of node n (core c, position pos = pos_lo[n]):
    #   ago row = c*128 + pos%128, 256B-slot = pos//128
    #   lo pass  (cores 0..3):  id = (c*128 + pos%128)*nranks + pos//128
    #   hi pass  (cores 4..7):  same with c-4
    n_core = np.arange(N) // npc  # owning core of each node
    tok_lo = ((n_core % lo_cut) * 128 + pos_lo % 128) * p.nranks + pos_lo // 128
    # dummy: any padded position (>= npc) holds zeros; use core 0/4, pos sec-1
    dummy_id = ((sec - 1) % 128 + 0 * 128) * p.nranks + (sec - 1) // 128

    # structure per (core, pass): collect cells, then uniformize across cores
    all_cells = [[None] * C for _ in range(2)]
    all_data = [[None] * C for _ in range(2)]
    for m in range(C):
        sel = d_core == m
        sm_ = src[sel]
        dl = dst[sel] - m * npc
        lo = s_core[sel] < lo_cut
        for half in range(2):
            emask = lo if half == 0 else ~lo
            es, ed = sm_[emask], dl[emask]
            if half == 0:
                jp = pos_lo[m * npc + ed]
            else:
                jp = pos_hi_local[m, ed]
            tks = tok_lo[es]
            # lo pass has the SL seed slot -> no dummy cover needed; hi pass
            # drains whole blocks from PSUM -> every position must be written.
            cells, data = _pass_structure(jp, tks, npc, dummy_id,
                                          need_cover=(half == 1))
            all_cells[half][m] = cells
            all_data[half][m] = data

    p.pass_chunks = []
    p.pass_cell_loc = []
    idx_arrays = [[None] * C for _ in range(2)]
    for half in range(2):
        cells_max = {}
        for m in range(C):
            for key, n in all_cells[half][m].items():
                cells_max[key] = max(cells_max.get(key, 0), n)
        chunks, cell_loc = _chunk_cells(cells_max, p.nblk)
        p.pass_chunks.append(chunks)
        p.pass_cell_loc.append(cell_loc)
        total = sum(chunks)
        for m in range(C):
            buf = np.full(total, dummy_id, np.int16)
            coff = np.concatenate([[0], np.cumsum(chunks)])
            for key, (ci, off, n) in cell_loc.items():
                d = all_data[half][m].get(key)
                if d is not None:
                    buf[coff[ci] + off: coff[ci] + off + len(d)] = d.astype(np.int16)
            idx_arrays[half][m] = buf
    p.idx_lo = [_pack_idx(idx_arrays[0][m]) for m in range(C)]
    p.idx_hi = [_pack_idx(idx_arrays[1][m]) for m in range(C)]

    # perm gather: work col i (lo pos i) = TS token pos_hi(node at lo pos i)
    p.idx_pm = []
    for m in range(C):
        pm = np.full(sec, npc, np.int64)  # pos npc..sec-1 of TS are zeros
        pm[:npc] = pos_hi_local[m, perm_lo[m]]
        p.idx_pm.append(_pack_idx(pm.astype(np.int16)))

    # per-core dense inputs
    p.xT = []
    p.d1 = []
    p.d2 = []
    p.d3 = []
    for m in range(C):
        pl = perm_lo[m]
        xm = np.asarray(x[m * npc:(m + 1) * npc][pl], dtype=np.float32)
        xt = np.zeros((p.nranks, 128, p.finc, 128), BF16)
        xv = xm.reshape(npc, p.finc, 128).astype(BF16)
        flat = xt.reshape(sec, p.finc, 128)
        flat[:npc] = xv
        p.xT.append(np.ascontiguousarray(xt.transpose(1, 0, 2, 3)))
        dv = np.zeros(sec, np.float32)
        dv[:npc] = dinv[m * npc + pl]
        dvt = dv.reshape(p.nranks, 128).T.copy()  # [128, nranks]
        p.d1.append(dvt)
        p.d2.append(dvt * dvt)
        p.d3.append(dvt * dvt * dvt)
    return p


# --------------------------------------------------------------------------
# Device kernel builder (one program, SPMD across 8 cores).
# --------------------------------------------------------------------------
def build_kernel(p, fmid, fout, sim_mode=False):
    dt = mybir.dt
    nc = bacc.Bacc("TRN2", num_swdge_queues=1)
    sec, nranks, npc, nblk = p.sec, p.nranks, p.npc, p.nblk
    llo = sum(p.pass_chunks[0])
    lhi = sum(p.pass_chunks[1])

    xT_d = nc.dram_tensor("xT", [128, nranks, p.finc, 128], dt.bfloat16, kind="ExternalInput")
    w1_d = nc.dram_tensor("w1", [128, p.finc, fmid], dt.bfloat16, kind="ExternalInput")
    w2_d = nc.dram_tensor("w2", [fmid, fmid], dt.bfloat16, kind="ExternalInput")
    w3_d = nc.dram_tensor("w3", [fmid, FOUT_PAD], dt.bfloat16, kind="ExternalInput")
    d1_d = nc.dram_tensor("d1", [128, nranks], dt.float32, kind="ExternalInput")
    d2_d = nc.dram_tensor("d2", [128, nranks], dt.float32, kind="ExternalInput")
    d3_d = nc.dram_tensor("d3", [128, nranks], dt.float32, kind="ExternalInput")
    id_d = nc.dram_tensor("ident", [128, 128], dt.bfloat16, kind="ExternalInput")
    il_d = nc.dram_tensor("idx_lo", [128, llo // 16], dt.int16, kind="ExternalInput")
    ih_d = nc.dram_tensor("idx_hi", [128, lhi // 16], dt.int16, kind="ExternalInput")
    ip_d = nc.dram_tensor("idx_pm", [128, sec // 16], dt.int16, kind="ExternalInput")
    out_d = nc.dram_tensor("out", [128, nranks * fout], dt.float32, kind="ExternalOutput")

    ag_in = nc.dram_tensor("ag_in", [128, sec], dt.bfloat16)
    ago = nc.dram_tensor("ago", [C * 128, sec], dt.bfloat16, addr_space="Shared")
    # flat [token, 128] views for the DRAM-source gathers
    ago_lo = ago[0:C // 2 * 128, :].rearrange("a (r f) -> (a r) f", f=128)
    ago_hi = ago[C // 2 * 128:C * 128, :].rearrange("a (r f) -> (a r) f", f=128)

    with tile.TileContext(nc) as tc:
        with (
            tc.tile_pool(name="main", bufs=1) as main,
            tc.tile_pool(name="mp", bufs=4) as mp,
            tc.tile_pool(name="gp", bufs=3) as gp,
            tc.tile_pool(name="psb", bufs=4, space=bass.MemorySpace.PSUM) as psb,
            tc.tile_pool(name="pss", bufs=2, space=bass.MemorySpace.PSUM) as pss,
        ):
            TS = main.tile([128, nranks, 128], dt.bfloat16)
            work = main.tile([128, 1, sec], dt.bfloat16)
            SL = main.tile([128, 1, sec], dt.float32)
            ident32 = main.tile([128, 128], dt.float32)
            pw = main.tile([128, 1, sec], dt.bfloat16)
            ident = main.tile([128, 128], dt.bfloat16)
            w1 = main.tile([128, p.finc, fmid], dt.bfloat16)
            w2 = main.tile([fmid, fmid], dt.bfloat16)
            w3 = main.tile([fmid, FOUT_PAD], dt.bfloat16)
            d1 = main.tile([128, nranks], dt.float32)
            d2 = main.tile([128, nranks], dt.float32)
            d3 = main.tile([128, nranks], dt.float32)
            il = main.tile([128, llo // 16], dt.int16)
            ih = main.tile([128, lhi // 16], dt.int16)
            ip = main.tile([128, sec // 16], dt.int16)

            nc.sync.dma_start(ident[:], id_d[:])
            nc.sync.dma_start(w1[:], w1_d[:])
            nc.sync.dma_start(w2[:], w2_d[:])
            nc.sync.dma_start(w3[:], w3_d[:])
            nc.sync.dma_start(d1[:], d1_d[:])
            nc.sync.dma_start(d2[:], d2_d[:])
            nc.sync.dma_start(d3[:], d3_d[:])
            nc.sync.dma_start(il[:], il_d[:])
            nc.sync.dma_start(ih[:], ih_d[:])
            nc.sync.dma_start(ip[:], ip_d[:])
            nc.vector.tensor_copy(ident32[:], ident[:])
            nc.vector.memset(TS[:], 0.0)
            nc.vector.memset(work[:], 0.0)
            nc.vector.memset(SL[:], 0.0)

            def pstage_l1():
                GRP = 13
                ngrp = (nranks + GRP - 1) // GRP
                xts = []
                for g in range(ngrp):
                    a, b = g * GRP, min(nranks, (g + 1) * GRP)
                    xt = mp.tile([128, b - a, p.finc, 128], dt.bfloat16, tag="m")
                    nc.sync.dma_start(xt[:], xT_d[:, a:b, :, :])
                    xts.append((a, xt))
                for c in range(nranks):
                    g = c // GRP
                    a, xt = xts[g]
                    ps = pss.tile([128, fmid], dt.float32, tag="pp")
                    for f in range(p.finc):
                        nc.tensor.matmul(ps[:], xt[:, c - a, f, :], w1[:, f, :],
                                         start=(f == 0), stop=(f == p.finc - 1))
                    nc.scalar.activation(work[:, 0, c * 128:(c + 1) * 128], ps[:],
                                         mybir.ActivationFunctionType.Copy,
                                         scale=d1[:, c:c + 1])
                    nc.scalar.activation(SL[:, 0, c * 128:(c + 1) * 128], ps[:],
                                         mybir.ActivationFunctionType.Copy,
                                         scale=d2[:, c:c + 1])

            def pstage(w, last=False):
                for c in range(nranks):
                    ps = pss.tile([128, w.shape[-1]], dt.float32, tag="pp")
                    nc.tensor.matmul(ps[:], work[:, 0, c * 128:(c + 1) * 128], w[:],
                                     start=True, stop=True)
                    psv = ps[:, 0:128] if w.shape[-1] >= 128 else ps[:]
                    nc.scalar.activation(work[:, 0, c * 128:(c + 1) * 128], psv,
                                         mybir.ActivationFunctionType.Copy,
                                         scale=d2[:, c:c + 1])
                    nc.scalar.activation(SL[:, 0, c * 128:(c + 1) * 128], psv,
                                         mybir.ActivationFunctionType.Copy,
                                         scale=d3[:, c:c + 1])

            def allgather():
                nc.sync.dma_start(ag_in[:, :], work[:, 0, :])
                if sim_mode:
                    # causal stand-in for the collective: a small write into ago
                    # ordered after the dense stage, so gathers wait like they
                    # would on the real collective
                    nc.sync.dma_start(ago[0:128, 0:128], work[:, 0, 0:128])
                else:
                    nc.gpsimd.collective_compute(
                        "AllGather", mybir.AluOpType.bypass,
                        replica_groups=[list(range(C))],
                        ins=[ag_in.ap().opt()], outs=[ago.ap().opt()])

            def gather_pass(half, idxt):
                """Issue the DRAM-source gathers for one pass; returns chunk
                tiles + locations."""
                chunks = p.pass_chunks[half]
                cell_loc = p.pass_cell_loc[half]
                in_ap = ago_lo if half == 0 else ago_hi
                coff = [0]
                for l in chunks:
                    coff.append(coff[-1] + l)
                mts = {}
                for ci, clen in enumerate(chunks):
                    m = gp.tile([128, 1, clen], dt.bfloat16, tag="g")
                    nc.gpsimd.dma_gather(
                        out_ap=m[:], in_ap=in_ap,
                        idxs_ap=idxt[:, coff[ci] // 16:(coff[ci] + clen) // 16],
                        num_idxs=clen, num_idxs_reg=clen, elem_size=128,
                        transpose=True, single_packet=False, queue_num=0)
                    mts[ci] = m
                return mts, cell_loc

            def reduction_hi(mts, cell_loc):
                """Hi pass slot matmuls, drain via transpose to TS
                (token-major, hi order)."""
                kmax = {}
                for (b, k) in cell_loc:
                    kmax[b] = max(kmax.get(b, -1), k)
                for b in range(nblk):
                    bsz = min(BLK, npc - b * BLK)
                    ps = psb.tile([128, BLK], dt.float32, tag="ps")
                    for k in range(kmax[b] + 1):
                        ci, off, n = cell_loc[(b, k)]
                        nc.tensor.matmul(ps[:, 0:n], ident[:], mts[ci][:, 0, off:off + n],
                                         start=(k == 0), stop=(k == kmax[b]))
                    sb = mp.tile([128, BLK], dt.bfloat16, tag="sb")
                    nc.scalar.activation(sb[:, 0:bsz], ps[:, 0:bsz],
                                         mybir.ActivationFunctionType.Copy)
                    for q in range((bsz + 127) // 128):
                        w_ = min(128, bsz - q * 128)
                        pt = pss.tile([128, 128], dt.bfloat16, tag="pt")
                        nc.tensor.transpose(pt[0:w_, :], sb[:, q * 128:q * 128 + w_],
                                            ident[:])
                        r = (b * BLK) // 128 + q
                        nc.scalar.activation(TS[0:w_, r, :], pt[0:w_, :],
                                             mybir.ActivationFunctionType.Copy)

            def perm_gather():
                nc.gpsimd.dma_gather(
                    out_ap=pw[:], in_ap=TS[:], idxs_ap=ip[:],
                    num_idxs=sec, num_idxs_reg=sec, elem_size=128,
                    transpose=True, sbuf_tokens_per_rank=128,
                    sbuf_free_dim_per_rank=256, sbuf_free_dim_pad_per_rank=0,
                    sbuf_byte_offset=0, single_packet=False, queue_num=0)

            def reduction_lo(mts, cell_loc, relu, final=False, sm=None, et=None):
                """Lo pass: SL seed + gathers + slot matmuls + permuted hi
                merge slot; drain (relu) straight into work. When final, the
                output transposes + exp run per block so the log_softmax tail
                overlaps the remaining aggregation."""
                kmax = {}
                for (b, k) in cell_loc:
                    kmax[b] = max(kmax.get(b, -1), k)
                for b in range(nblk):
                    a0 = b * BLK
                    bsz = min(BLK, npc - a0)
                    ps = psb.tile([128, BLK], dt.float32, tag="ps")
                    # seed = transposed SL chunks (token-major -> drain layout);
                    # SL's pad columns are zero, so full 128-wide chunks are safe
                    for q in range((bsz + 127) // 128):
                        nc.tensor.matmul(
                            ps[:, q * 128:(q + 1) * 128],
                            SL[:, 0, a0 + q * 128:a0 + (q + 1) * 128],
                            ident32[:], is_transpose=True, start=True, stop=False)
                    for k in range(kmax.get(b, -1) + 1):
                        ci, off, n = cell_loc[(b, k)]
                        nc.tensor.matmul(ps[:, 0:n], ident[:], mts[ci][:, 0, off:off + n],
                                         start=False, stop=False)
                    nc.tensor.matmul(ps[:, 0:bsz], ident[:], pw[:, 0, a0:a0 + bsz],
                                     start=False, stop=True)
                    fn = (mybir.ActivationFunctionType.Relu if relu
                          else mybir.ActivationFunctionType.Copy)
                    nc.scalar.activation(work[:, 0, a0:a0 + bsz], ps[:, 0:bsz], fn)
                    if final:
                        for c in range(4 * b, min(4 * b + 4, nranks)):
                            pt = pss.tile([128, 128], dt.bfloat16, tag="pt")
                            nc.tensor.transpose(pt[:], work[:, 0, c * 128:(c + 1) * 128],
                                                ident[:])
                            nc.scalar.activation(sm[:, c, :], pt[:, 0:fout],
                                                 mybir.ActivationFunctionType.Copy,
                                                 scale=d1[:, c:c + 1])
                            nc.scalar.activation(et[:, c, :], sm[:, c, :],
                                                 mybir.ActivationFunctionType.Exp)

            def output_finish(sm, et):
                lg = mp.tile([128, nranks], dt.float32, tag="o")
                nc.vector.reduce_sum(lg[:], et[:], axis=mybir.AxisListType.X)
                nc.scalar.activation(lg[:], lg[:], mybir.ActivationFunctionType.Ln)
                for c in range(nranks):
                    nc.vector.tensor_scalar_sub(sm[:, c, :], sm[:, c, :], lg[:, c:c + 1])
                nc.sync.dma_start(out_d[:, :], sm[:].rearrange("q c f -> q (c f)"))

            # ---- program ----
            pstage_l1()
            sm = mp.tile([128, nranks, fout], dt.float32, tag="o")
            et = mp.tile([128, nranks, fout], dt.float32, tag="o")
            for layer in range(3):
                allgather()
                mts_hi, cl_hi = gather_pass(1, ih)
                mts_lo, cl_lo = gather_pass(0, il)
                reduction_hi(mts_hi, cl_hi)
                perm_gather()
                if layer < 2:
                    reduction_lo(mts_lo, cl_lo, relu=True)
                    pstage(w2 if layer == 0 else w3)
                else:
                    reduction_lo(mts_lo, cl_lo, relu=False, final=True, sm=sm, et=et)
                    output_finish(sm, et)
    nc.compile()
    return nc


# --------------------------------------------------------------------------
# Entry point
# --------------------------------------------------------------------------
def _make_in_maps(p, inputs, fmid, fout):
    W_in = np.asarray(inputs["W_in"], dtype=np.float32)
    W_mid = np.asarray(inputs["W_mid"], dtype=np.float32)
    W_out = np.asarray(inputs["W_out"], dtype=np.float32)
    w1 = np.ascontiguousarray(
        W_in.reshape(p.finc, 128, fmid).transpose(1, 0, 2).astype(BF16))
    w2 = np.ascontiguousarray(W_mid.astype(BF16))
    w3 = np.zeros((fmid, FOUT_PAD), BF16)
    w3[:, :fout] = W_out.astype(BF16)
    ident = np.eye(128, dtype=np.float32).astype(BF16)
    in_maps = []
    for m in range(C):
        in_maps.append({
            "xT": p.xT[m].reshape(128, p.nranks, p.finc, 128),
            "w1": w1, "w2": w2, "w3": w3,
            "d1": p.d1[m], "d2": p.d2[m], "d3": p.d3[m], "ident": ident,
            "idx_lo": p.idx_lo[m], "idx_hi": p.idx_hi[m], "idx_pm": p.idx_pm[m],
        })
    return in_maps


def _run(inputs, trace=False, trace_cores=None):
    x = np.asarray(inputs["x"], dtype=np.float32)
    edge_index = np.asarray(inputs["edge_index"])
    W_in = np.asarray(inputs["W_in"], dtype=np.float32)
    W_out = np.asarray(inputs["W_out"], dtype=np.float32)
    for bname in ("b_in", "b_mid", "b_out"):
        if np.any(np.asarray(inputs[bname])):
            raise NotImplementedError("nonzero bias path not implemented")

    N, fin = x.shape
    fmid = W_in.shape[1]
    fout = W_out.shape[1]
    p = preprocess(x, edge_index, N)

    nc = build_kernel(p, fmid, fout)

    in_maps = _make_in_maps(p, inputs, fmid, fout)
    kw = {}
    if trace:
        kw = dict(trace=True, trace_cores=trace_cores or [0])
    r = run_bass_kernel_spmd(nc, in_maps, core_ids=list(range(C)), **kw)

    out = np.empty((N, fout), np.float32)
    for m in range(C):
        res = r.results[m]["out"]  # [128, nranks*fout] partition-major
        rows = res.reshape(128, p.nranks, fout).transpose(1, 0, 2).reshape(p.sec, fout)
        out[m * p.npc + p.perm_lo[m]] = rows[:p.npc]
    return out, r


def kernel(**inputs) -> np.ndarray:
    out, _ = _run(inputs)
    return out


# revision 13
# speedup vs baseline: 1.0060x; 1.0060x over previous
"""3-layer GCN (GCNConv x3, PyG defaults) on 8 Trainium2 NeuronCores.

Strategy (graph/data parallel, per sharding hint):
  - Nodes are sharded 8 ways by destination range (6250 nodes/core, padded to
    6272-token sections of 128-feat bf16 tokens, 256 B each).
  - Per layer, one AllGather collective publishes every core's freshly
    computed p-rows (p = dinv * h) to a shared DRAM buffer `ago`
    [C*128, nranks*128]; token (core c, pos) sits at row c*128 + pos%128,
    byte offset (pos//128)*256 -- i.e. 256 B contiguous in DRAM.
  - A = D^-1/2 (A+I) D^-1/2 aggregation: per-edge gathers run on the GPSIMD
    dma_gather DIRECTLY from `ago` DRAM (no SBUF staging copy of the full
    token space); the segment-sum runs on the TensorEngine as identity-matmul
    accumulation into PSUM, slot-major with nodes sorted by degree descending
    (slot k covers the prefix of nodes with degree > k).
  - dma_gather indices are int16, so sources split into two passes by source
    core (cores 0-3 / 4-7, 25088 tokens each, re-based in_ap); the hi pass
    drains (transposed) into a token-major tile TS, which a small on-chip
    gather permutes into the lo pass's node order; the permuted tile and the
    self-loop term are merged into the lo pass's PSUM accumulation as extra
    identity-matmul slots, and the drain applies relu directly.
  - Self-loops never go through the gather: the dense-stage PSUM drain emits
    a second copy scaled by dinv^2 into an SBUF tile SL used as the PSUM
    seed slot.
  - Dense stages (X@W, act@W) run on the TensorEngine; deg^-1/2 scales fold
    into per-partition scalars of the PSUM-drain activation op.
All 8 cores run one identical program; only input data differs per core.
"""
import sys
import os

sys.path.insert(0, "/opt/trn_rl_repo")

import numpy as np
import ml_dtypes

from concourse import bass, bacc, mybir
from concourse import tile
from concourse.bass_utils import run_bass_kernel_spmd

BF16 = ml_dtypes.bfloat16
C = 8
BLK = 512
CHUNK = 8192
FOUT_PAD = 128  # W_out columns padded so layer-3 tokens share the 256B layout


# --------------------------------------------------------------------------
# Host-side preprocessing: pure integer/index work + normalization constants.
# --------------------------------------------------------------------------
class Plan:
    pass


def _pack_idx(vals):
    """int16 list -> [128, len/16] wrapped (i -> [i%16, i//16]) replicated x8."""
    n = len(vals)
    assert n % 16 == 0
    a = np.asarray(vals, dtype=np.int16).reshape(n // 16, 16).T  # [16, n/16]
    return np.tile(a, (8, 1))


def _pass_structure(jpos, toks, npc, dummy_tok, need_cover):
    """Slot-major structure for one (core, pass).

    jpos: position (by this pass's degree-desc order) of each edge's dst.
    toks: gather token id of each edge's src.
    need_cover: if True, positions with zero degree get one dummy edge so
    every position is written by some slot (required when the PSUM block has
    no seed slot).
    Returns (cells, data) where cells[(b, k)] = count and data[(b, k)] = token
    array (dst positions ascending within each cell).
    """
    cnt = np.bincount(jpos, minlength=npc)
    if need_cover:
        zpos = np.nonzero(cnt == 0)[0]
        if len(zpos):
            jpos = np.concatenate([jpos, zpos])
            toks = np.concatenate([toks, np.full(len(zpos), dummy_tok, np.int64)])
            cnt[zpos] = 1
    order = np.argsort(jpos, kind="stable")
    js = jpos[order]
    ts = toks[order]
    starts = np.zeros(npc, np.int64)
    starts[1:] = np.cumsum(cnt)[:-1]
    kk = np.arange(len(js)) - starts[js]
    bb = js // BLK
    o2 = np.lexsort((js, kk, bb))
    js, ts, kk, bb = js[o2], ts[o2], kk[o2], bb[o2]
    cells = {}
    data = {}
    cell_id = bb * 4096 + kk
    uniq, first = np.unique(cell_id, return_index=True)
    bounds = list(first) + [len(cell_id)]
    for i, u in enumerate(uniq):
        b, k = int(u) // 4096, int(u) % 4096
        seg = slice(bounds[i], bounds[i + 1])
        cells[(b, k)] = bounds[i + 1] - bounds[i]
        data[(b, k)] = ts[seg]
    return cells, data


def _chunk_cells(cells_max, nblk):
    """Pack (b,k) cells into gather chunks of <= CHUNK columns.

    Returns (chunks, cell_loc): chunks = list of padded lengths;
    cell_loc[(b,k)] = (chunk_idx, offset, n).
    """
    chunks = []
    cell_loc = {}
    cur = 0
    for b in range(nblk):
        ks = sorted(k for (bb, k) in cells_max if bb == b)
        for k in ks:
            n = cells_max[(b, k)]
            cap = 2048 if len(chunks) <= 1 else CHUNK
            if cur == 0 or chunks[-1] + n > cap:
                chunks.append(0)
                cur = 1
            cell_loc[(b, k)] = (len(chunks) - 1, chunks[-1], n)
            chunks[-1] += n
    chunks = [(l + 127) // 128 * 128 for l in chunks]
    return chunks, cell_loc


def preprocess(x, edge_index, n_nodes):
    p = Plan()
    N = n_nodes
    assert N % C == 0
    npc = N // C
    sec = (npc + 127) // 128 * 128
    p.npc, p.sec = npc, sec
    p.nranks = sec // 128
    p.nblk = (npc + BLK - 1) // BLK
    p.fin = x.shape[1]
    assert p.fin % 128 == 0
    p.finc = p.fin // 128

    src = np.asarray(edge_index[0], dtype=np.int64)
    dst = np.asarray(edge_index[1], dtype=np.int64)
    deg = (np.bincount(dst, minlength=N) + 1).astype(np.float32)
    dinv = (1.0 / np.sqrt(deg)).astype(np.float32)

    # NO self-loops in the edge lists: the dense stage emits the dinv^2-scaled
    # self term directly (SL tile).
    s_core = src // npc
    d_core = dst // npc
    lo_cut = C // 2

    # per-core degree split and orders
    perm_lo = np.empty((C, npc), np.int64)
    perm_hi = np.empty((C, npc), np.int64)
    pos_lo = np.empty(N, np.int64)
    pos_hi_local = np.empty((C, npc), np.int64)
    for m in range(C):
        sel = d_core == m
        dl = dst[sel] - m * npc
        lo = s_core[sel] < lo_cut
        cl = np.bincount(dl[lo], minlength=npc)
        ch = np.bincount(dl[~lo], minlength=npc)
        pl = np.argsort(-np.maximum(cl, 1), kind="stable")
        ph = np.argsort(-np.maximum(ch, 1), kind="stable")
        perm_lo[m], perm_hi[m] = pl, ph
        pos_lo[m * npc + pl] = np.arange(npc)
        pos_hi_local[m, ph] = np.arange(npc)
    p.perm_lo = perm_lo

    # DRAM token id of node n (core c, position pos = pos_lo[n]):
    #   ago row = c*128 + pos%128, 256B-slot = pos//128
    #   lo pass  (cores 0..3):  id = (c*128 + pos%128)*nranks + pos//128
    #   hi pass  (cores 4..7):  same with c-4
    n_core = np.arange(N) // npc  # owning core of each node
    tok_lo = ((n_core % lo_cut) * 128 + pos_lo % 128) * p.nranks + pos_lo // 128
    # dummy: any padded position (>= npc) holds zeros; use core 0/4, pos sec-1
    dummy_id = ((sec - 1) % 128 + 0 * 128) * p.nranks + (sec - 1) // 128

    # structure per (core, pass): collect cells, then uniformize across cores
    all_cells = [[None] * C for _ in range(2)]
    all_data = [[None] * C for _ in range(2)]
    for m in range(C):
        sel = d_core == m
        sm_ = src[sel]
        dl = dst[sel] - m * npc
        lo = s_core[sel] < lo_cut
        for half in range(2):
            emask = lo if half == 0 else ~lo
            es, ed = sm_[emask], dl[emask]
            if half == 0:
                jp = pos_lo[m * npc + ed]
            else:
                jp = pos_hi_local[m, ed]
            tks = tok_lo[es]
            # lo pass has the SL seed slot -> no dummy cover needed; hi pass
            # drains whole blocks from PSUM -> every position must be written.
            cells, data = _pass_structure(jp, tks, npc, dummy_id,
                                          need_cover=(half == 1))
            all_cells[half][m] = cells
            all_data[half][m] = data

    p.pass_chunks = []
    p.pass_cell_loc = []
    idx_arrays = [[None] * C for _ in range(2)]
    for half in range(2):
        cells_max = {}
        for m in range(C):
            for key, n in all_cells[half][m].items():
                cells_max[key] = max(cells_max.get(key, 0), n)
        chunks, cell_loc = _chunk_cells(cells_max, p.nblk)
        p.pass_chunks.append(chunks)
        p.pass_cell_loc.append(cell_loc)
        total = sum(chunks)
        for m in range(C):
            buf = np.full(total, dummy_id, np.int16)
            coff = np.concatenate([[0], np.cumsum(chunks)])
            for key, (ci, off, n) in cell_loc.items():
                d = all_data[half][m].get(key)
                if d is not None:
                    buf[coff[ci] + off: coff[ci] + off + len(d)] = d.astype(np.int16)
            idx_arrays[half][m] = buf
    p.idx_lo = [_pack_idx(idx_arrays[0][m]) for m in range(C)]
    p.idx_hi = [_pack_idx(idx_arrays[1][m]) for m in range(C)]

    # perm gather: work col i (lo pos i) = TS token pos_hi(node at lo pos i)
    p.idx_pm = []
    for m in range(C):
        pm = np.full(sec, npc, np.int64)  # pos npc..sec-1 of TS are zeros
        pm[:npc] = pos_hi_local[m, perm_lo[m]]
        p.idx_pm.append(_pack_idx(pm.astype(np.int16)))

    # per-core dense inputs
    p.xT = []
    p.d1 = []
    p.d2 = []
    p.d3 = []
    for m in range(C):
        pl = perm_lo[m]
        xm = np.asarray(x[m * npc:(m + 1) * npc][pl], dtype=np.float32)
        xt = np.zeros((p.nranks, 128, p.finc, 128), BF16)
        xv = xm.reshape(npc, p.finc, 128).astype(BF16)
        flat = xt.reshape(sec, p.finc, 128)
        flat[:npc] = xv
        p.xT.append(np.ascontiguousarray(xt.transpose(1, 0, 2, 3)))
        dv = np.zeros(sec, np.float32)
        dv[:npc] = dinv[m * npc + pl]
        dvt = dv.reshape(p.nranks, 128).T.copy()  # [128, nranks]
        p.d1.append(dvt)
        p.d2.append(dvt * dvt)
        p.d3.append(dvt * dvt * dvt)
    return p


# --------------------------------------------------------------------------
# Device kernel builder (one program, SPMD across 8 cores).
# --------------------------------------------------------------------------
def build_kernel(p, fmid, fout, sim_mode=False):
    dt = mybir.dt
    nc = bacc.Bacc("TRN2", num_swdge_queues=1)
    sec, nranks, npc, nblk = p.sec, p.nranks, p.npc, p.nblk
    llo = sum(p.pass_chunks[0])
    lhi = sum(p.pass_chunks[1])

    xT_d = nc.dram_tensor("xT", [128, nranks, p.finc, 128], dt.bfloat16, kind="ExternalInput")
    w1_d = nc.dram_tensor("w1", [128, p.finc, fmid], dt.bfloat16, kind="ExternalInput")
    w2_d = nc.dram_tensor("w2", [fmid, fmid], dt.bfloat16, kind="ExternalInput")
    w3_d = nc.dram_tensor("w3", [fmid, FOUT_PAD], dt.bfloat16, kind="ExternalInput")
    d1_d = nc.dram_tensor("d1", [128, nranks], dt.float32, kind="ExternalInput")
    d2_d = nc.dram_tensor("d2", [128, nranks], dt.float32, kind="ExternalInput")
    d3_d = nc.dram_tensor("d3", [128, nranks], dt.float32, kind="ExternalInput")
    id_d = nc.dram_tensor("ident", [128, 128], dt.bfloat16, kind="ExternalInput")
    il_d = nc.dram_tensor("idx_lo", [128, llo // 16], dt.int16, kind="ExternalInput")
    ih_d = nc.dram_tensor("idx_hi", [128, lhi // 16], dt.int16, kind="ExternalInput")
    ip_d = nc.dram_tensor("idx_pm", [128, sec // 16], dt.int16, kind="ExternalInput")
    out_d = nc.dram_tensor("out", [128, nranks * fout], dt.float32, kind="ExternalOutput")

    ag_in = nc.dram_tensor("ag_in", [128, sec], dt.bfloat16)
    ago = nc.dram_tensor("ago", [C * 128, sec], dt.bfloat16, addr_space="Shared")
    # flat [token, 128] views for the DRAM-source gathers
    ago_lo = ago[0:C // 2 * 128, :].rearrange("a (r f) -> (a r) f", f=128)
    ago_hi = ago[C // 2 * 128:C * 128, :].rearrange("a (r f) -> (a r) f", f=128)

    with tile.TileContext(nc) as tc:
        with (
            tc.tile_pool(name="main", bufs=1) as main,
            tc.tile_pool(name="mp", bufs=4) as mp,
            tc.tile_pool(name="gp", bufs=3) as gp,
            tc.tile_pool(name="psb", bufs=4, space=bass.MemorySpace.PSUM) as psb,
            tc.tile_pool(name="pss", bufs=2, space=bass.MemorySpace.PSUM) as pss,
        ):
            TS = main.tile([128, nranks, 128], dt.bfloat16)
            sA = main.tile([128, 1, sec], dt.bfloat16)
            work = main.tile([128, 1, sec], dt.bfloat16)
            SL = main.tile([128, 1, sec], dt.float32)
            ident32 = main.tile([128, 128], dt.float32)
            pw = main.tile([128, 1, sec], dt.bfloat16)
            ident = main.tile([128, 128], dt.bfloat16)
            w1 = main.tile([128, p.finc, fmid], dt.bfloat16)
            w2 = main.tile([fmid, fmid], dt.bfloat16)
            w3 = main.tile([fmid, FOUT_PAD], dt.bfloat16)
            d1 = main.tile([128, nranks], dt.float32)
            d2 = main.tile([128, nranks], dt.float32)
            d3 = main.tile([128, nranks], dt.float32)
            il = main.tile([128, llo // 16], dt.int16)
            ih = main.tile([128, lhi // 16], dt.int16)
            ip = main.tile([128, sec // 16], dt.int16)

            nc.sync.dma_start(ident[:], id_d[:])
            nc.sync.dma_start(w1[:], w1_d[:])
            nc.sync.dma_start(w2[:], w2_d[:])
            nc.sync.dma_start(w3[:], w3_d[:])
            nc.sync.dma_start(d1[:], d1_d[:])
            nc.sync.dma_start(d2[:], d2_d[:])
            nc.sync.dma_start(d3[:], d3_d[:])
            nc.sync.dma_start(il[:], il_d[:])
            nc.sync.dma_start(ih[:], ih_d[:])
            nc.sync.dma_start(ip[:], ip_d[:])
            nc.vector.tensor_copy(ident32[:], ident[:])
            nc.vector.memset(TS[:], 0.0)
            nc.vector.memset(work[:], 0.0)
            nc.vector.memset(SL[:], 0.0)

            def pstage_l1():
                GRP = 13
                ngrp = (nranks + GRP - 1) // GRP
                xts = []
                for g in range(ngrp):
                    a, b = g * GRP, min(nranks, (g + 1) * GRP)
                    xt = mp.tile([128, b - a, p.finc, 128], dt.bfloat16, tag="m")
                    nc.sync.dma_start(xt[:], xT_d[:, a:b, :, :])
                    xts.append((a, xt))
                for c in range(nranks):
                    g = c // GRP
                    a, xt = xts[g]
                    ps = pss.tile([128, fmid], dt.float32, tag="pp")
                    for f in range(p.finc):
                        nc.tensor.matmul(ps[:], xt[:, c - a, f, :], w1[:, f, :],
                                         start=(f == 0), stop=(f == p.finc - 1))
                    nc.scalar.activation(work[:, 0, c * 128:(c + 1) * 128], ps[:],
                                         mybir.ActivationFunctionType.Copy,
                                         scale=d1[:, c:c + 1])
                    nc.scalar.activation(SL[:, 0, c * 128:(c + 1) * 128], ps[:],
                                         mybir.ActivationFunctionType.Copy,
                                         scale=d2[:, c:c + 1])

            def allgather():
                if sim_mode:
                    # causal stand-in for the collective: own shard lands in
                    # ago, ordered after the complete dense stage
                    nc.sync.dma_start(ago[0:128, :], work[:, 0, :])
                else:
                    nc.gpsimd.collective_compute(
                        "AllGather", mybir.AluOpType.bypass,
                        replica_groups=[list(range(C))],
                        ins=[ag_in.ap().opt()], outs=[ago.ap().opt()])

            def gather_pass(half, idxt):
                """Issue the DRAM-source gathers for one pass; returns chunk
                tiles + locations."""
                chunks = p.pass_chunks[half]
                cell_loc = p.pass_cell_loc[half]
                in_ap = ago_lo if half == 0 else ago_hi
                coff = [0]
                for l in chunks:
                    coff.append(coff[-1] + l)
                mts = {}
                for ci, clen in enumerate(chunks):
                    m = gp.tile([128, 1, clen], dt.bfloat16, tag="g")
                    nc.gpsimd.dma_gather(
                        out_ap=m[:], in_ap=in_ap,
                        idxs_ap=idxt[:, coff[ci] // 16:(coff[ci] + clen) // 16],
                        num_idxs=clen, num_idxs_reg=clen, elem_size=128,
                        transpose=True, single_packet=False, queue_num=0)
                    mts[ci] = m
                return mts, cell_loc

            def reduction_hi(mts, cell_loc):
                """Hi pass slot matmuls, drain via transpose to TS
                (token-major, hi order)."""
                kmax = {}
                for (b, k) in cell_loc:
                    kmax[b] = max(kmax.get(b, -1), k)
                for b in range(nblk):
                    bsz = min(BLK, npc - b * BLK)
                    ps = psb.tile([128, BLK], dt.float32, tag="ps")
                    for k in range(kmax[b] + 1):
                        ci, off, n = cell_loc[(b, k)]
                        nc.tensor.matmul(ps[:, 0:n], ident[:], mts[ci][:, 0, off:off + n],
                                         start=(k == 0), stop=(k == kmax[b]))
                    sb = mp.tile([128, BLK], dt.bfloat16, tag="sb")
                    nc.scalar.activation(sb[:, 0:bsz], ps[:, 0:bsz],
                                         mybir.ActivationFunctionType.Copy)
                    for q in range((bsz + 127) // 128):
                        w_ = min(128, bsz - q * 128)
                        pt = pss.tile([128, 128], dt.bfloat16, tag="pt")
                        nc.tensor.transpose(pt[0:w_, :], sb[:, q * 128:q * 128 + w_],
                                            ident[:])
                        r = (b * BLK) // 128 + q
                        nc.scalar.activation(TS[0:w_, r, :], pt[0:w_, :],
                                             mybir.ActivationFunctionType.Copy)

            def perm_gather():
                nc.gpsimd.dma_gather(
                    out_ap=pw[:], in_ap=TS[:], idxs_ap=ip[:],
                    num_idxs=sec, num_idxs_reg=sec, elem_size=128,
                    transpose=True, sbuf_tokens_per_rank=128,
                    sbuf_free_dim_per_rank=256, sbuf_free_dim_pad_per_rank=0,
                    sbuf_byte_offset=0, single_packet=False, queue_num=0)

            def reduction_lo(mts, cell_loc):
                """Lo pass: SL seed + slot matmuls per block; drain to sA
                with no dependency on the perm result (keeps PSUM recycling
                while gathers stream)."""
                kmax = {}
                for (b, k) in cell_loc:
                    kmax[b] = max(kmax.get(b, -1), k)
                for b in range(nblk):
                    a0 = b * BLK
                    bsz = min(BLK, npc - a0)
                    ps = psb.tile([128, BLK], dt.float32, tag="ps")
                    # seed = transposed SL chunks (token-major -> drain layout);
                    # SL's pad columns are zero, so full 128-wide chunks are safe
                    nslot = kmax.get(b, -1) + 1
                    nq = (bsz + 127) // 128
                    for q in range(nq):
                        nc.tensor.matmul(
                            ps[:, q * 128:(q + 1) * 128],
                            SL[:, 0, a0 + q * 128:a0 + (q + 1) * 128],
                            ident32[:], is_transpose=True, start=True,
                            stop=(nslot == 0 and q == nq - 1))
                    for k in range(nslot):
                        ci, off, n = cell_loc[(b, k)]
                        nc.tensor.matmul(ps[:, 0:n], ident[:], mts[ci][:, 0, off:off + n],
                                         start=False, stop=(k == nslot - 1))
                    nc.scalar.activation(sA[:, 0, a0:a0 + bsz], ps[:, 0:bsz],
                                         mybir.ActivationFunctionType.Copy)

            def merge_layer(relu, w=None, final=False, sm=None, et=None, lg=None):
                """Per block: work = (relu of) sA + pw, then the follow-on
                stage for the block's ranks (dense pstage or the output
                pipeline), so layer transitions stream block by block."""
                for b in range(nblk):
                    a0 = b * BLK
                    bsz = min(BLK, npc - a0)
                    nc.vector.tensor_tensor(work[:, 0, a0:a0 + bsz],
                                            sA[:, 0, a0:a0 + bsz],
                                            pw[:, 0, a0:a0 + bsz],
                                            mybir.AluOpType.add)
                    if relu:
                        nc.scalar.activation(work[:, 0, a0:a0 + bsz],
                                             work[:, 0, a0:a0 + bsz],
                                             mybir.ActivationFunctionType.Relu)
                    ranks = range(4 * b, min(4 * b + 4, nranks))
                    if not final:
                        for c in ranks:
                            ps = pss.tile([128, w.shape[-1]], dt.float32, tag="pp")
                            nc.tensor.matmul(ps[:], work[:, 0, c * 128:(c + 1) * 128],
                                             w[:], start=True, stop=True)
                            psv = ps[:, 0:128] if w.shape[-1] >= 128 else ps[:]
                            nc.scalar.activation(work[:, 0, c * 128:(c + 1) * 128], psv,
                                                 mybir.ActivationFunctionType.Copy,
                                                 scale=d2[:, c:c + 1])
                            nc.scalar.activation(SL[:, 0, c * 128:(c + 1) * 128], psv,
                                                 mybir.ActivationFunctionType.Copy,
                                                 scale=d3[:, c:c + 1])
                        nc.sync.dma_start(
                            ag_in[:, 4 * b * 128:min(4 * b + 4, nranks) * 128],
                            work[:, 0, 4 * b * 128:min(4 * b + 4, nranks) * 128])
                    else:
                        for c in ranks:
                            pt = pss.tile([128, 128], dt.bfloat16, tag="pt")
                            nc.tensor.transpose(pt[:], work[:, 0, c * 128:(c + 1) * 128],
                                                ident[:])
                            nc.scalar.activation(sm[:, c, :], pt[:, 0:fout],
                                                 mybir.ActivationFunctionType.Copy,
                                                 scale=d1[:, c:c + 1])
                            nc.scalar.activation(et[:, c, :], sm[:, c, :],
                                                 mybir.ActivationFunctionType.Exp)
                        cr0, cr1 = 4 * b, min(4 * b + 4, nranks)
                        nc.vector.reduce_sum(lg[:, cr0:cr1], et[:, cr0:cr1, :],
                                             axis=mybir.AxisListType.X)
                        nc.scalar.activation(lg[:, cr0:cr1], lg[:, cr0:cr1],
                                             mybir.ActivationFunctionType.Ln)
                        for c in range(cr0, cr1):
                            nc.vector.tensor_scalar_sub(sm[:, c, :], sm[:, c, :],
                                                        lg[:, c:c + 1])
                        nc.sync.dma_start(
                            out_d[:, cr0 * fout:cr1 * fout],
                            sm[:, cr0:cr1, :].rearrange("q c f -> q (c f)"))

            # ---- program ----
            pstage_l1()
            nc.sync.dma_start(ag_in[:, :], work[:, 0, :])
            sm = mp.tile([128, nranks, fout], dt.float32, tag="o")
            et = mp.tile([128, nranks, fout], dt.float32, tag="o")
            lg = mp.tile([128, nranks], dt.float32, tag="o")
            for layer in range(3):
                allgather()
                mts_hi, cl_hi = gather_pass(1, ih)
                mts_lo, cl_lo = gather_pass(0, il)
                reduction_hi(mts_hi, cl_hi)
                perm_gather()
                reduction_lo(mts_lo, cl_lo)
                if layer < 2:
                    merge_layer(relu=True, w=(w2 if layer == 0 else w3))
                else:
                    merge_layer(relu=False, final=True, sm=sm, et=et, lg=lg)
    nc.compile()
    return nc


# --------------------------------------------------------------------------
# Entry point
# --------------------------------------------------------------------------
def _make_in_maps(p, inputs, fmid, fout):
    W_in = np.asarray(inputs["W_in"], dtype=np.float32)
    W_mid = np.asarray(inputs["W_mid"], dtype=np.float32)
    W_out = np.asarray(inputs["W_out"], dtype=np.float32)
    w1 = np.ascontiguousarray(
        W_in.reshape(p.finc, 128, fmid).transpose(1, 0, 2).astype(BF16))
    w2 = np.ascontiguousarray(W_mid.astype(BF16))
    w3 = np.zeros((fmid, FOUT_PAD), BF16)
    w3[:, :fout] = W_out.astype(BF16)
    ident = np.eye(128, dtype=np.float32).astype(BF16)
    in_maps = []
    for m in range(C):
        in_maps.append({
            "xT": p.xT[m].reshape(128, p.nranks, p.finc, 128),
            "w1": w1, "w2": w2, "w3": w3,
            "d1": p.d1[m], "d2": p.d2[m], "d3": p.d3[m], "ident": ident,
            "idx_lo": p.idx_lo[m], "idx_hi": p.idx_hi[m], "idx_pm": p.idx_pm[m],
        })
    return in_maps


def _run(inputs, trace=False, trace_cores=None):
    x = np.asarray(inputs["x"], dtype=np.float32)
    edge_index = np.asarray(inputs["edge_index"])
    W_in = np.asarray(inputs["W_in"], dtype=np.float32)
    W_out = np.asarray(inputs["W_out"], dtype=np.float32)
    for bname in ("b_in", "b_mid", "b_out"):
        if np.any(np.asarray(inputs[bname])):
            raise NotImplementedError("nonzero bias path not implemented")

    N, fin = x.shape
    fmid = W_in.shape[1]
    fout = W_out.shape[1]
    p = preprocess(x, edge_index, N)

    nc = build_kernel(p, fmid, fout)

    in_maps = _make_in_maps(p, inputs, fmid, fout)
    kw = {}
    if trace:
        kw = dict(trace=True, trace_cores=trace_cores or [0])
    r = run_bass_kernel_spmd(nc, in_maps, core_ids=list(range(C)), **kw)

    out = np.empty((N, fout), np.float32)
    for m in range(C):
        res = r.results[m]["out"]  # [128, nranks*fout] partition-major
        rows = res.reshape(128, p.nranks, fout).transpose(1, 0, 2).reshape(p.sec, fout)
        out[m * p.npc + p.perm_lo[m]] = rows[:p.npc]
    return out, r


def kernel(**inputs) -> np.ndarray:
    out, _ = _run(inputs)
    return out


# revision 14
# speedup vs baseline: 1.0145x; 1.0085x over previous
"""3-layer GCN (GCNConv x3, PyG defaults) on 8 Trainium2 NeuronCores.

Strategy (graph/data parallel, per sharding hint):
  - Nodes are sharded 8 ways by destination range (6250 nodes/core, padded to
    6272-token sections of 128-feat bf16 tokens, 256 B each).
  - Per layer, one AllGather collective publishes every core's freshly
    computed p-rows (p = dinv * h) to a shared DRAM buffer `ago`
    [C*128, nranks*128]; token (core c, pos) sits at row c*128 + pos%128,
    byte offset (pos//128)*256 -- i.e. 256 B contiguous in DRAM.
  - A = D^-1/2 (A+I) D^-1/2 aggregation: per-edge gathers run on the GPSIMD
    dma_gather DIRECTLY from `ago` DRAM (no SBUF staging copy of the full
    token space); the segment-sum runs on the TensorEngine as identity-matmul
    accumulation into PSUM, slot-major with nodes sorted by degree descending
    (slot k covers the prefix of nodes with degree > k).
  - dma_gather indices are int16, so sources split into two passes by source
    core (cores 0-3 / 4-7, 25088 tokens each, re-based in_ap); the hi pass
    drains (transposed) into a token-major tile TS, which a small on-chip
    gather permutes into the lo pass's node order; the permuted tile and the
    self-loop term are merged into the lo pass's PSUM accumulation as extra
    identity-matmul slots, and the drain applies relu directly.
  - Self-loops never go through the gather: the dense-stage PSUM drain emits
    a second copy scaled by dinv^2 into an SBUF tile SL used as the PSUM
    seed slot.
  - Dense stages (X@W, act@W) run on the TensorEngine; deg^-1/2 scales fold
    into per-partition scalars of the PSUM-drain activation op.
All 8 cores run one identical program; only input data differs per core.
"""
import sys
import os

sys.path.insert(0, "/opt/trn_rl_repo")

import numpy as np
import ml_dtypes

from concourse import bass, bacc, mybir
from concourse import tile
from concourse.bass_utils import run_bass_kernel_spmd

BF16 = ml_dtypes.bfloat16
C = 8
BLK = 512
CHUNK = 8192
FOUT_PAD = 128  # W_out columns padded so layer-3 tokens share the 256B layout


# --------------------------------------------------------------------------
# Host-side preprocessing: pure integer/index work + normalization constants.
# --------------------------------------------------------------------------
class Plan:
    pass


def _pack_idx(vals):
    """int16 list -> [128, len/16] wrapped (i -> [i%16, i//16]) replicated x8."""
    n = len(vals)
    assert n % 16 == 0
    a = np.asarray(vals, dtype=np.int16).reshape(n // 16, 16).T  # [16, n/16]
    return np.tile(a, (8, 1))


def _pass_structure(jpos, toks, npc, dummy_tok, need_cover):
    """Slot-major structure for one (core, pass).

    jpos: position (by this pass's degree-desc order) of each edge's dst.
    toks: gather token id of each edge's src.
    need_cover: if True, positions with zero degree get one dummy edge so
    every position is written by some slot (required when the PSUM block has
    no seed slot).
    Returns (cells, data) where cells[(b, k)] = count and data[(b, k)] = token
    array (dst positions ascending within each cell).
    """
    cnt = np.bincount(jpos, minlength=npc)
    if need_cover:
        zpos = np.nonzero(cnt == 0)[0]
        if len(zpos):
            jpos = np.concatenate([jpos, zpos])
            toks = np.concatenate([toks, np.full(len(zpos), dummy_tok, np.int64)])
            cnt[zpos] = 1
    order = np.argsort(jpos, kind="stable")
    js = jpos[order]
    ts = toks[order]
    starts = np.zeros(npc, np.int64)
    starts[1:] = np.cumsum(cnt)[:-1]
    kk = np.arange(len(js)) - starts[js]
    bb = js // BLK
    o2 = np.lexsort((js, kk, bb))
    js, ts, kk, bb = js[o2], ts[o2], kk[o2], bb[o2]
    cells = {}
    data = {}
    cell_id = bb * 4096 + kk
    uniq, first = np.unique(cell_id, return_index=True)
    bounds = list(first) + [len(cell_id)]
    for i, u in enumerate(uniq):
        b, k = int(u) // 4096, int(u) % 4096
        seg = slice(bounds[i], bounds[i + 1])
        cells[(b, k)] = bounds[i + 1] - bounds[i]
        data[(b, k)] = ts[seg]
    return cells, data


def _chunk_cells(cells_max, nblk):
    """Pack (b,k) cells into gather chunks of <= CHUNK columns.

    Returns (chunks, cell_loc): chunks = list of padded lengths;
    cell_loc[(b,k)] = (chunk_idx, offset, n).
    """
    chunks = []
    cell_loc = {}
    cur = 0
    for b in range(nblk):
        ks = sorted(k for (bb, k) in cells_max if bb == b)
        for k in ks:
            n = cells_max[(b, k)]
            cap = 2048 if len(chunks) <= 1 else CHUNK
            if cur == 0 or chunks[-1] + n > cap:
                chunks.append(0)
                cur = 1
            cell_loc[(b, k)] = (len(chunks) - 1, chunks[-1], n)
            chunks[-1] += n
    chunks = [(l + 127) // 128 * 128 for l in chunks]
    return chunks, cell_loc


def preprocess(x, edge_index, n_nodes):
    p = Plan()
    N = n_nodes
    assert N % C == 0
    npc = N // C
    sec = (npc + 127) // 128 * 128
    p.npc, p.sec = npc, sec
    p.nranks = sec // 128
    p.nblk = (npc + BLK - 1) // BLK
    p.fin = x.shape[1]
    assert p.fin % 128 == 0
    p.finc = p.fin // 128

    src = np.asarray(edge_index[0], dtype=np.int64)
    dst = np.asarray(edge_index[1], dtype=np.int64)
    deg = (np.bincount(dst, minlength=N) + 1).astype(np.float32)
    dinv = (1.0 / np.sqrt(deg)).astype(np.float32)

    # NO self-loops in the edge lists: the dense stage emits the dinv^2-scaled
    # self term directly (SL tile).
    s_core = src // npc
    d_core = dst // npc
    lo_cut = C // 2

    # per-core degree split and orders
    perm_lo = np.empty((C, npc), np.int64)
    perm_hi = np.empty((C, npc), np.int64)
    pos_lo = np.empty(N, np.int64)
    pos_hi_local = np.empty((C, npc), np.int64)
    for m in range(C):
        sel = d_core == m
        dl = dst[sel] - m * npc
        lo = s_core[sel] < lo_cut
        cl = np.bincount(dl[lo], minlength=npc)
        ch = np.bincount(dl[~lo], minlength=npc)
        pl = np.argsort(-np.maximum(cl, 1), kind="stable")
        ph = np.argsort(-np.maximum(ch, 1), kind="stable")
        perm_lo[m], perm_hi[m] = pl, ph
        pos_lo[m * npc + pl] = np.arange(npc)
        pos_hi_local[m, ph] = np.arange(npc)
    p.perm_lo = perm_lo

    # DRAM token id of node n (core c, position pos = pos_lo[n]):
    #   ago row = c*128 + pos%128, 256B-slot = pos//128
    #   lo pass  (cores 0..3):  id = (c*128 + pos%128)*nranks + pos//128
    #   hi pass  (cores 4..7):  same with c-4
    n_core = np.arange(N) // npc  # owning core of each node
    tok_lo = ((n_core % lo_cut) * 128 + pos_lo % 128) * p.nranks + pos_lo // 128
    # dummy: any padded position (>= npc) holds zeros; use core 0/4, pos sec-1
    dummy_id = ((sec - 1) % 128 + 0 * 128) * p.nranks + (sec - 1) // 128

    # structure per (core, pass): collect cells, then uniformize across cores
    all_cells = [[None] * C for _ in range(2)]
    all_data = [[None] * C for _ in range(2)]
    for m in range(C):
        sel = d_core == m
        sm_ = src[sel]
        dl = dst[sel] - m * npc
        lo = s_core[sel] < lo_cut
        for half in range(2):
            emask = lo if half == 0 else ~lo
            es, ed = sm_[emask], dl[emask]
            if half == 0:
                jp = pos_lo[m * npc + ed]
            else:
                jp = pos_hi_local[m, ed]
            tks = tok_lo[es]
            # lo pass has the SL seed slot -> no dummy cover needed; hi pass
            # drains whole blocks from PSUM -> every position must be written.
            cells, data = _pass_structure(jp, tks, npc, dummy_id,
                                          need_cover=(half == 1))
            all_cells[half][m] = cells
            all_data[half][m] = data

    p.pass_chunks = []
    p.pass_cell_loc = []
    idx_arrays = [[None] * C for _ in range(2)]
    for half in range(2):
        cells_max = {}
        for m in range(C):
            for key, n in all_cells[half][m].items():
                cells_max[key] = max(cells_max.get(key, 0), n)
        chunks, cell_loc = _chunk_cells(cells_max, p.nblk)
        p.pass_chunks.append(chunks)
        p.pass_cell_loc.append(cell_loc)
        total = sum(chunks)
        for m in range(C):
            buf = np.full(total, dummy_id, np.int16)
            coff = np.concatenate([[0], np.cumsum(chunks)])
            for key, (ci, off, n) in cell_loc.items():
                d = all_data[half][m].get(key)
                if d is not None:
                    buf[coff[ci] + off: coff[ci] + off + len(d)] = d.astype(np.int16)
            idx_arrays[half][m] = buf
    p.idx_lo = [_pack_idx(idx_arrays[0][m]) for m in range(C)]
    p.idx_hi = [_pack_idx(idx_arrays[1][m]) for m in range(C)]

    # perm gather: work col i (lo pos i) = TS token pos_hi(node at lo pos i)
    p.idx_pm = []
    for m in range(C):
        pm = np.full(sec, npc, np.int64)  # pos npc..sec-1 of TS are zeros
        pm[:npc] = pos_hi_local[m, perm_lo[m]]
        p.idx_pm.append(_pack_idx(pm.astype(np.int16)))

    # per-core dense inputs
    p.xT = []
    p.d1 = []
    p.d2 = []
    p.d3 = []
    for m in range(C):
        pl = perm_lo[m]
        xm = np.asarray(x[m * npc:(m + 1) * npc][pl], dtype=np.float32)
        xt = np.zeros((p.nranks, 128, p.finc, 128), BF16)
        xv = xm.reshape(npc, p.finc, 128).astype(BF16)
        flat = xt.reshape(sec, p.finc, 128)
        flat[:npc] = xv
        p.xT.append(np.ascontiguousarray(xt.transpose(1, 0, 2, 3)))
        dv = np.zeros(sec, np.float32)
        dv[:npc] = dinv[m * npc + pl]
        dvt = dv.reshape(p.nranks, 128).T.copy()  # [128, nranks]
        p.d1.append(dvt)
        p.d2.append(dvt * dvt)
        p.d3.append(dvt * dvt * dvt)
    return p


# --------------------------------------------------------------------------
# Device kernel builder (one program, SPMD across 8 cores).
# --------------------------------------------------------------------------
def build_kernel(p, fmid, fout, sim_mode=False):
    dt = mybir.dt
    nc = bacc.Bacc("TRN2", num_swdge_queues=1)
    sec, nranks, npc, nblk = p.sec, p.nranks, p.npc, p.nblk
    llo = sum(p.pass_chunks[0])
    lhi = sum(p.pass_chunks[1])

    xT_d = nc.dram_tensor("xT", [128, nranks, p.finc, 128], dt.bfloat16, kind="ExternalInput")
    w1_d = nc.dram_tensor("w1", [128, p.finc, fmid], dt.bfloat16, kind="ExternalInput")
    w2_d = nc.dram_tensor("w2", [fmid, fmid], dt.bfloat16, kind="ExternalInput")
    w3_d = nc.dram_tensor("w3", [fmid, FOUT_PAD], dt.bfloat16, kind="ExternalInput")
    d1_d = nc.dram_tensor("d1", [128, nranks], dt.float32, kind="ExternalInput")
    d2_d = nc.dram_tensor("d2", [128, nranks], dt.float32, kind="ExternalInput")
    d3_d = nc.dram_tensor("d3", [128, nranks], dt.float32, kind="ExternalInput")
    id_d = nc.dram_tensor("ident", [128, 128], dt.bfloat16, kind="ExternalInput")
    il_d = nc.dram_tensor("idx_lo", [128, llo // 16], dt.int16, kind="ExternalInput")
    ih_d = nc.dram_tensor("idx_hi", [128, lhi // 16], dt.int16, kind="ExternalInput")
    ip_d = nc.dram_tensor("idx_pm", [128, sec // 16], dt.int16, kind="ExternalInput")
    out_d = nc.dram_tensor("out", [128, nranks * fout], dt.float32, kind="ExternalOutput")

    ag_in = nc.dram_tensor("ag_in", [128, sec], dt.bfloat16)
    ago = nc.dram_tensor("ago", [C * 128, sec], dt.bfloat16, addr_space="Shared")
    # flat [token, 128] views for the DRAM-source gathers
    ago_lo = ago[0:C // 2 * 128, :].rearrange("a (r f) -> (a r) f", f=128)
    ago_hi = ago[C // 2 * 128:C * 128, :].rearrange("a (r f) -> (a r) f", f=128)

    with tile.TileContext(nc) as tc:
        with (
            tc.tile_pool(name="main", bufs=1) as main,
            tc.tile_pool(name="mp", bufs=4) as mp,
            tc.tile_pool(name="gp", bufs=3) as gp,
            tc.tile_pool(name="psb", bufs=4, space=bass.MemorySpace.PSUM) as psb,
            tc.tile_pool(name="pss", bufs=2, space=bass.MemorySpace.PSUM) as pss,
        ):
            TS = main.tile([128, nranks, 128], dt.bfloat16)
            sA = main.tile([128, 1, sec], dt.bfloat16)
            work = main.tile([128, 1, sec], dt.bfloat16)
            SL = main.tile([128, 1, sec], dt.float32)
            ident32 = main.tile([128, 128], dt.float32)
            pw = main.tile([128, 1, sec], dt.bfloat16)
            ident = main.tile([128, 128], dt.bfloat16)
            w1 = main.tile([128, p.finc, fmid], dt.bfloat16)
            w2 = main.tile([fmid, fmid], dt.bfloat16)
            w3 = main.tile([fmid, FOUT_PAD], dt.bfloat16)
            d1 = main.tile([128, nranks], dt.float32)
            d2 = main.tile([128, nranks], dt.float32)
            d3 = main.tile([128, nranks], dt.float32)
            il = main.tile([128, llo // 16], dt.int16)
            ih = main.tile([128, lhi // 16], dt.int16)
            ip = main.tile([128, sec // 16], dt.int16)

            nc.sync.dma_start(ident[:], id_d[:])
            nc.sync.dma_start(w1[:], w1_d[:])
            nc.sync.dma_start(w2[:], w2_d[:])
            nc.sync.dma_start(w3[:], w3_d[:])
            nc.sync.dma_start(d1[:], d1_d[:])
            nc.sync.dma_start(d2[:], d2_d[:])
            nc.sync.dma_start(d3[:], d3_d[:])
            nc.sync.dma_start(il[:], il_d[:])
            nc.sync.dma_start(ih[:], ih_d[:])
            nc.sync.dma_start(ip[:], ip_d[:])
            nc.vector.tensor_copy(ident32[:], ident[:])
            nc.vector.memset(TS[:], 0.0)
            nc.vector.memset(work[:], 0.0)
            nc.vector.memset(SL[:], 0.0)

            def pstage_l1():
                GRP = 13
                ngrp = (nranks + GRP - 1) // GRP
                xts = []
                for g in range(ngrp):
                    a, b = g * GRP, min(nranks, (g + 1) * GRP)
                    xt = mp.tile([128, b - a, p.finc, 128], dt.bfloat16, tag="m")
                    nc.sync.dma_start(xt[:], xT_d[:, a:b, :, :])
                    xts.append((a, xt))
                for c in range(nranks):
                    g = c // GRP
                    a, xt = xts[g]
                    ps = pss.tile([128, fmid], dt.float32, tag="pp")
                    for f in range(p.finc):
                        nc.tensor.matmul(ps[:], xt[:, c - a, f, :], w1[:, f, :],
                                         start=(f == 0), stop=(f == p.finc - 1))
                    nc.vector.tensor_scalar_mul(work[:, 0, c * 128:(c + 1) * 128],
                                                ps[:], d1[:, c:c + 1])
                    nc.vector.tensor_scalar_mul(SL[:, 0, c * 128:(c + 1) * 128],
                                                ps[:], d2[:, c:c + 1])

            def allgather():
                if sim_mode:
                    # causal stand-in for the collective: own shard lands in
                    # ago, ordered after the complete dense stage
                    nc.sync.dma_start(ago[0:128, :], work[:, 0, :])
                else:
                    nc.gpsimd.collective_compute(
                        "AllGather", mybir.AluOpType.bypass,
                        replica_groups=[list(range(C))],
                        ins=[ag_in.ap().opt()], outs=[ago.ap().opt()])

            def gather_pass(half, idxt):
                """Issue the DRAM-source gathers for one pass; returns chunk
                tiles + locations."""
                chunks = p.pass_chunks[half]
                cell_loc = p.pass_cell_loc[half]
                in_ap = ago_lo if half == 0 else ago_hi
                coff = [0]
                for l in chunks:
                    coff.append(coff[-1] + l)
                mts = {}
                for ci, clen in enumerate(chunks):
                    m = gp.tile([128, 1, clen], dt.bfloat16, tag="g")
                    nc.gpsimd.dma_gather(
                        out_ap=m[:], in_ap=in_ap,
                        idxs_ap=idxt[:, coff[ci] // 16:(coff[ci] + clen) // 16],
                        num_idxs=clen, num_idxs_reg=clen, elem_size=128,
                        transpose=True, single_packet=False, queue_num=0)
                    mts[ci] = m
                return mts, cell_loc

            def reduction_hi(mts, cell_loc):
                """Hi pass slot matmuls, drain via transpose to TS
                (token-major, hi order)."""
                kmax = {}
                for (b, k) in cell_loc:
                    kmax[b] = max(kmax.get(b, -1), k)
                for b in range(nblk):
                    bsz = min(BLK, npc - b * BLK)
                    ps = psb.tile([128, BLK], dt.float32, tag="ps")
                    for k in range(kmax[b] + 1):
                        ci, off, n = cell_loc[(b, k)]
                        nc.tensor.matmul(ps[:, 0:n], ident[:], mts[ci][:, 0, off:off + n],
                                         start=(k == 0), stop=(k == kmax[b]))
                    sb = mp.tile([128, BLK], dt.bfloat16, tag="sb")
                    nc.vector.tensor_copy(sb[:, 0:bsz], ps[:, 0:bsz])
                    for q in range((bsz + 127) // 128):
                        w_ = min(128, bsz - q * 128)
                        pt = pss.tile([128, 128], dt.bfloat16, tag="pt")
                        nc.tensor.transpose(pt[0:w_, :], sb[:, q * 128:q * 128 + w_],
                                            ident[:])
                        r = (b * BLK) // 128 + q
                        nc.vector.tensor_copy(TS[0:w_, r, :], pt[0:w_, :])

            def perm_gather():
                nc.gpsimd.dma_gather(
                    out_ap=pw[:], in_ap=TS[:], idxs_ap=ip[:],
                    num_idxs=sec, num_idxs_reg=sec, elem_size=128,
                    transpose=True, sbuf_tokens_per_rank=128,
                    sbuf_free_dim_per_rank=256, sbuf_free_dim_pad_per_rank=0,
                    sbuf_byte_offset=0, single_packet=False, queue_num=0)

            def reduction_lo(mts, cell_loc):
                """Lo pass: SL seed + slot matmuls per block; drain to sA
                with no dependency on the perm result (keeps PSUM recycling
                while gathers stream)."""
                kmax = {}
                for (b, k) in cell_loc:
                    kmax[b] = max(kmax.get(b, -1), k)
                for b in range(nblk):
                    a0 = b * BLK
                    bsz = min(BLK, npc - a0)
                    ps = psb.tile([128, BLK], dt.float32, tag="ps")
                    # seed = transposed SL chunks (token-major -> drain layout);
                    # SL's pad columns are zero, so full 128-wide chunks are safe
                    nslot = kmax.get(b, -1) + 1
                    nq = (bsz + 127) // 128
                    for q in range(nq):
                        nc.tensor.matmul(
                            ps[:, q * 128:(q + 1) * 128],
                            SL[:, 0, a0 + q * 128:a0 + (q + 1) * 128],
                            ident32[:], is_transpose=True, start=True,
                            stop=(nslot == 0 and q == nq - 1))
                    for k in range(nslot):
                        ci, off, n = cell_loc[(b, k)]
                        nc.tensor.matmul(ps[:, 0:n], ident[:], mts[ci][:, 0, off:off + n],
                                         start=False, stop=(k == nslot - 1))
                    nc.vector.tensor_copy(sA[:, 0, a0:a0 + bsz], ps[:, 0:bsz])

            def merge_layer(relu, w=None, final=False, sm=None, et=None, lg=None):
                """Per block: work = (relu of) sA + pw, then the follow-on
                stage for the block's ranks (dense pstage or the output
                pipeline), so layer transitions stream block by block."""
                for b in range(nblk):
                    a0 = b * BLK
                    bsz = min(BLK, npc - a0)
                    nc.vector.tensor_tensor(work[:, 0, a0:a0 + bsz],
                                            sA[:, 0, a0:a0 + bsz],
                                            pw[:, 0, a0:a0 + bsz],
                                            mybir.AluOpType.add)
                    if relu:
                        nc.vector.tensor_scalar_max(work[:, 0, a0:a0 + bsz],
                                                    work[:, 0, a0:a0 + bsz], 0.0)
                    ranks = range(4 * b, min(4 * b + 4, nranks))
                    if not final:
                        for c in ranks:
                            ps = pss.tile([128, w.shape[-1]], dt.float32, tag="pp")
                            nc.tensor.matmul(ps[:], work[:, 0, c * 128:(c + 1) * 128],
                                             w[:], start=True, stop=True)
                            psv = ps[:, 0:128] if w.shape[-1] >= 128 else ps[:]
                            nc.vector.tensor_scalar_mul(
                                work[:, 0, c * 128:(c + 1) * 128], psv, d2[:, c:c + 1])
                            nc.vector.tensor_scalar_mul(
                                SL[:, 0, c * 128:(c + 1) * 128], psv, d3[:, c:c + 1])
                        nc.sync.dma_start(
                            ag_in[:, 4 * b * 128:min(4 * b + 4, nranks) * 128],
                            work[:, 0, 4 * b * 128:min(4 * b + 4, nranks) * 128])
                    else:
                        for c in ranks:
                            pt = pss.tile([128, fout], dt.bfloat16, tag="pt")
                            nc.tensor.matmul(pt[:], work[:, 0, c * 128:(c + 1) * 128],
                                             ident[:, 0:fout], is_transpose=True,
                                             start=True, stop=True)
                            nc.vector.tensor_scalar_mul(sm[:, c, :], pt[:],
                                                        d1[:, c:c + 1])
                            nc.scalar.activation(et[:, c, :], sm[:, c, :],
                                                 mybir.ActivationFunctionType.Exp)
                        cr0, cr1 = 4 * b, min(4 * b + 4, nranks)
                        nc.vector.reduce_sum(lg[:, cr0:cr1], et[:, cr0:cr1, :],
                                             axis=mybir.AxisListType.X)
                        nc.scalar.activation(lg[:, cr0:cr1], lg[:, cr0:cr1],
                                             mybir.ActivationFunctionType.Ln)
                        for c in range(cr0, cr1):
                            nc.vector.tensor_scalar_sub(sm[:, c, :], sm[:, c, :],
                                                        lg[:, c:c + 1])
                        nc.sync.dma_start(
                            out_d[:, cr0 * fout:cr1 * fout],
                            sm[:, cr0:cr1, :].rearrange("q c f -> q (c f)"))

            # ---- program ----
            pstage_l1()
            nc.sync.dma_start(ag_in[:, :], work[:, 0, :])
            sm = mp.tile([128, nranks, fout], dt.float32, tag="o")
            et = mp.tile([128, nranks, fout], dt.float32, tag="o")
            lg = mp.tile([128, nranks], dt.float32, tag="o")
            for layer in range(3):
                allgather()
                mts_hi, cl_hi = gather_pass(1, ih)
                mts_lo, cl_lo = gather_pass(0, il)
                reduction_hi(mts_hi, cl_hi)
                perm_gather()
                reduction_lo(mts_lo, cl_lo)
                if layer < 2:
                    merge_layer(relu=True, w=(w2 if layer == 0 else w3))
                else:
                    merge_layer(relu=False, final=True, sm=sm, et=et, lg=lg)
    nc.compile()
    return nc


# --------------------------------------------------------------------------
# Entry point
# --------------------------------------------------------------------------
def _make_in_maps(p, inputs, fmid, fout):
    W_in = np.asarray(inputs["W_in"], dtype=np.float32)
    W_mid = np.asarray(inputs["W_mid"], dtype=np.float32)
    W_out = np.asarray(inputs["W_out"], dtype=np.float32)
    w1 = np.ascontiguousarray(
        W_in.reshape(p.finc, 128, fmid).transpose(1, 0, 2).astype(BF16))
    w2 = np.ascontiguousarray(W_mid.astype(BF16))
    w3 = np.zeros((fmid, FOUT_PAD), BF16)
    w3[:, :fout] = W_out.astype(BF16)
    ident = np.eye(128, dtype=np.float32).astype(BF16)
    in_maps = []
    for m in range(C):
        in_maps.append({
            "xT": p.xT[m].reshape(128, p.nranks, p.finc, 128),
            "w1": w1, "w2": w2, "w3": w3,
            "d1": p.d1[m], "d2": p.d2[m], "d3": p.d3[m], "ident": ident,
            "idx_lo": p.idx_lo[m], "idx_hi": p.idx_hi[m], "idx_pm": p.idx_pm[m],
        })
    return in_maps


def _run(inputs, trace=False, trace_cores=None):
    x = np.asarray(inputs["x"], dtype=np.float32)
    edge_index = np.asarray(inputs["edge_index"])
    W_in = np.asarray(inputs["W_in"], dtype=np.float32)
    W_out = np.asarray(inputs["W_out"], dtype=np.float32)
    for bname in ("b_in", "b_mid", "b_out"):
        if np.any(np.asarray(inputs[bname])):
            raise NotImplementedError("nonzero bias path not implemented")

    N, fin = x.shape
    fmid = W_in.shape[1]
    fout = W_out.shape[1]
    p = preprocess(x, edge_index, N)

    nc = build_kernel(p, fmid, fout)

    in_maps = _make_in_maps(p, inputs, fmid, fout)
    kw = {}
    if trace:
        kw = dict(trace=True, trace_cores=trace_cores or [0])
    r = run_bass_kernel_spmd(nc, in_maps, core_ids=list(range(C)), **kw)

    out = np.empty((N, fout), np.float32)
    for m in range(C):
        res = r.results[m]["out"]  # [128, nranks*fout] partition-major
        rows = res.reshape(128, p.nranks, fout).transpose(1, 0, 2).reshape(p.sec, fout)
        out[m * p.npc + p.perm_lo[m]] = rows[:p.npc]
    return out, r


def kernel(**inputs) -> np.ndarray:
    out, _ = _run(inputs)
    return out


# revision 15
# speedup vs baseline: 1.0575x; 1.0424x over previous
"""3-layer GCN (GCNConv x3, PyG defaults) on 8 Trainium2 NeuronCores.

Strategy (graph/data parallel, per sharding hint):
  - Nodes are sharded 8 ways by destination range (6250 nodes/core, padded to
    6272-token sections of 128-feat bf16 tokens, 256 B each).
  - Per layer, one AllGather collective publishes every core's freshly
    computed p-rows (p = dinv * h) to a shared DRAM buffer `ago`
    [C*128, nranks*128]; token (core c, pos) sits at row c*128 + pos%128,
    byte offset (pos//128)*256 -- i.e. 256 B contiguous in DRAM.
  - A = D^-1/2 (A+I) D^-1/2 aggregation: per-edge gathers run on the GPSIMD
    dma_gather DIRECTLY from `ago` DRAM (no SBUF staging copy of the full
    token space); the segment-sum runs on the TensorEngine as identity-matmul
    accumulation into PSUM, slot-major with nodes sorted by degree descending
    (slot k covers the prefix of nodes with degree > k).
  - dma_gather indices are int16, so sources split into two passes by source
    core (cores 0-3 / 4-7, 25088 tokens each, re-based in_ap); the hi pass
    drains (transposed) into a token-major tile TS, which a small on-chip
    gather permutes into the lo pass's node order; the permuted tile and the
    self-loop term are merged into the lo pass's PSUM accumulation as extra
    identity-matmul slots, and the drain applies relu directly.
  - Self-loops never go through the gather: the dense-stage PSUM drain emits
    a second copy scaled by dinv^2 into an SBUF tile SL used as the PSUM
    seed slot.
  - Dense stages (X@W, act@W) run on the TensorEngine; deg^-1/2 scales fold
    into per-partition scalars of the PSUM-drain activation op.
All 8 cores run one identical program; only input data differs per core.
"""
import sys
import os

sys.path.insert(0, "/opt/trn_rl_repo")

import numpy as np
import ml_dtypes

from concourse import bass, bacc, mybir
from concourse import tile
from concourse.bass_utils import run_bass_kernel_spmd

BF16 = ml_dtypes.bfloat16
C = 8
BLK = 512
CHUNK = 8192
FOUT_PAD = 128  # W_out columns padded so layer-3 tokens share the 256B layout


# --------------------------------------------------------------------------
# Host-side preprocessing: pure integer/index work + normalization constants.
# --------------------------------------------------------------------------
class Plan:
    pass


def _pack_idx(vals):
    """int16 list -> [128, len/16] wrapped (i -> [i%16, i//16]) replicated x8."""
    n = len(vals)
    assert n % 16 == 0
    a = np.asarray(vals, dtype=np.int16).reshape(n // 16, 16).T  # [16, n/16]
    return np.tile(a, (8, 1))


def _pass_structure(jpos, toks, npc, dummy_tok, need_cover):
    """Slot-major structure for one (core, pass).

    jpos: position (by this pass's degree-desc order) of each edge's dst.
    toks: gather token id of each edge's src.
    need_cover: if True, positions with zero degree get one dummy edge so
    every position is written by some slot (required when the PSUM block has
    no seed slot).
    Returns (cells, data) where cells[(b, k)] = count and data[(b, k)] = token
    array (dst positions ascending within each cell).
    """
    cnt = np.bincount(jpos, minlength=npc)
    if need_cover:
        zpos = np.nonzero(cnt == 0)[0]
        if len(zpos):
            jpos = np.concatenate([jpos, zpos])
            toks = np.concatenate([toks, np.full(len(zpos), dummy_tok, np.int64)])
            cnt[zpos] = 1
    order = np.argsort(jpos, kind="stable")
    js = jpos[order]
    ts = toks[order]
    starts = np.zeros(npc, np.int64)
    starts[1:] = np.cumsum(cnt)[:-1]
    kk = np.arange(len(js)) - starts[js]
    bb = js // BLK
    o2 = np.lexsort((js, kk, bb))
    js, ts, kk, bb = js[o2], ts[o2], kk[o2], bb[o2]
    cells = {}
    data = {}
    cell_id = bb * 4096 + kk
    uniq, first = np.unique(cell_id, return_index=True)
    bounds = list(first) + [len(cell_id)]
    for i, u in enumerate(uniq):
        b, k = int(u) // 4096, int(u) % 4096
        seg = slice(bounds[i], bounds[i + 1])
        cells[(b, k)] = bounds[i + 1] - bounds[i]
        data[(b, k)] = ts[seg]
    return cells, data


def _chunk_cells(cells_max, nblk):
    """Pack (b,k) cells into gather chunks of <= CHUNK columns.

    Returns (chunks, cell_loc): chunks = list of padded lengths;
    cell_loc[(b,k)] = (chunk_idx, offset, n).
    """
    chunks = []
    cell_loc = {}
    cur = 0
    for b in range(nblk):
        ks = sorted(k for (bb, k) in cells_max if bb == b)
        for k in ks:
            n = cells_max[(b, k)]
            cap = 2048 if len(chunks) <= 1 else CHUNK
            if cur == 0 or chunks[-1] + n > cap:
                chunks.append(0)
                cur = 1
            cell_loc[(b, k)] = (len(chunks) - 1, chunks[-1], n)
            chunks[-1] += n
    chunks = [(l + 127) // 128 * 128 for l in chunks]
    return chunks, cell_loc


def preprocess(x, edge_index, n_nodes):
    p = Plan()
    N = n_nodes
    assert N % C == 0
    npc = N // C
    sec = (npc + 127) // 128 * 128
    p.npc, p.sec = npc, sec
    p.nranks = sec // 128
    p.nblk = (npc + BLK - 1) // BLK
    p.fin = x.shape[1]
    assert p.fin % 128 == 0
    p.finc = p.fin // 128

    src = np.asarray(edge_index[0], dtype=np.int64)
    dst = np.asarray(edge_index[1], dtype=np.int64)
    deg = (np.bincount(dst, minlength=N) + 1).astype(np.float32)
    dinv = (1.0 / np.sqrt(deg)).astype(np.float32)

    # NO self-loops in the edge lists: the dense stage emits the dinv^2-scaled
    # self term directly (SL tile).
    s_core = src // npc
    d_core = dst // npc
    lo_cut = C // 2

    # per-core degree split and orders
    perm_lo = np.empty((C, npc), np.int64)
    perm_hi = np.empty((C, npc), np.int64)
    pos_lo = np.empty(N, np.int64)
    pos_hi_local = np.empty((C, npc), np.int64)
    for m in range(C):
        sel = d_core == m
        dl = dst[sel] - m * npc
        lo = s_core[sel] < lo_cut
        cl = np.bincount(dl[lo], minlength=npc)
        ch = np.bincount(dl[~lo], minlength=npc)
        pl = np.argsort(-np.maximum(cl, 1), kind="stable")
        ph = np.argsort(-np.maximum(ch, 1), kind="stable")
        perm_lo[m], perm_hi[m] = pl, ph
        pos_lo[m * npc + pl] = np.arange(npc)
        pos_hi_local[m, ph] = np.arange(npc)
    p.perm_lo = perm_lo

    # DRAM token id of node n (core c, position pos = pos_lo[n]):
    #   ago row = c*128 + pos%128, 256B-slot = pos//128
    #   lo pass  (cores 0..3):  id = (c*128 + pos%128)*nranks + pos//128
    #   hi pass  (cores 4..7):  same with c-4
    n_core = np.arange(N) // npc  # owning core of each node
    tok_lo = ((n_core % lo_cut) * 128 + pos_lo % 128) * p.nranks + pos_lo // 128
    # dummy: any padded position (>= npc) holds zeros; use core 0/4, pos sec-1
    dummy_id = ((sec - 1) % 128 + 0 * 128) * p.nranks + (sec - 1) // 128

    # structure per (core, pass): collect cells, then uniformize across cores
    all_cells = [[None] * C for _ in range(2)]
    all_data = [[None] * C for _ in range(2)]
    for m in range(C):
        sel = d_core == m
        sm_ = src[sel]
        dl = dst[sel] - m * npc
        lo = s_core[sel] < lo_cut
        for half in range(2):
            emask = lo if half == 0 else ~lo
            es, ed = sm_[emask], dl[emask]
            if half == 0:
                jp = pos_lo[m * npc + ed]
            else:
                jp = pos_hi_local[m, ed]
            tks = tok_lo[es]
            # lo pass has the SL seed slot -> no dummy cover needed; hi pass
            # drains whole blocks from PSUM -> every position must be written.
            cells, data = _pass_structure(jp, tks, npc, dummy_id,
                                          need_cover=(half == 1))
            all_cells[half][m] = cells
            all_data[half][m] = data

    p.pass_chunks = []
    p.pass_cell_loc = []
    idx_arrays = [[None] * C for _ in range(2)]
    for half in range(2):
        cells_max = {}
        for m in range(C):
            for key, n in all_cells[half][m].items():
                cells_max[key] = max(cells_max.get(key, 0), n)
        chunks, cell_loc = _chunk_cells(cells_max, p.nblk)
        p.pass_chunks.append(chunks)
        p.pass_cell_loc.append(cell_loc)
        total = sum(chunks)
        for m in range(C):
            buf = np.full(total, dummy_id, np.int16)
            coff = np.concatenate([[0], np.cumsum(chunks)])
            for key, (ci, off, n) in cell_loc.items():
                d = all_data[half][m].get(key)
                if d is not None:
                    buf[coff[ci] + off: coff[ci] + off + len(d)] = d.astype(np.int16)
            idx_arrays[half][m] = buf
    p.idx_lo = [_pack_idx(idx_arrays[0][m]) for m in range(C)]
    p.idx_hi = [_pack_idx(idx_arrays[1][m]) for m in range(C)]

    # perm gather: work col i (lo pos i) = TS token pos_hi(node at lo pos i)
    p.idx_pm = []
    for m in range(C):
        pm = np.full(sec, npc, np.int64)  # pos npc..sec-1 of TS are zeros
        pm[:npc] = pos_hi_local[m, perm_lo[m]]
        p.idx_pm.append(_pack_idx(pm.astype(np.int16)))

    # per-core dense inputs
    p.xT = []
    p.d1 = []
    p.d2 = []
    p.d3 = []
    for m in range(C):
        pl = perm_lo[m]
        xm = np.asarray(x[m * npc:(m + 1) * npc][pl], dtype=np.float32)
        xt = np.zeros((p.nranks, 128, p.finc, 128), BF16)
        xv = xm.reshape(npc, p.finc, 128).astype(BF16)
        flat = xt.reshape(sec, p.finc, 128)
        flat[:npc] = xv
        p.xT.append(np.ascontiguousarray(xt.transpose(1, 0, 2, 3)))
        dv = np.zeros(sec, np.float32)
        dv[:npc] = dinv[m * npc + pl]
        dvt = dv.reshape(p.nranks, 128).T.copy()  # [128, nranks]
        p.d1.append(dvt)
        p.d2.append(dvt * dvt)
        p.d3.append(dvt * dvt * dvt)
    return p


# --------------------------------------------------------------------------
# Device kernel builder (one program, SPMD across 8 cores).
# --------------------------------------------------------------------------
def build_kernel(p, fmid, fout, sim_mode=False):
    dt = mybir.dt
    nc = bacc.Bacc("TRN2", num_swdge_queues=1)
    sec, nranks, npc, nblk = p.sec, p.nranks, p.npc, p.nblk
    llo = sum(p.pass_chunks[0])
    lhi = sum(p.pass_chunks[1])

    xT_d = nc.dram_tensor("xT", [128, nranks, p.finc, 128], dt.bfloat16, kind="ExternalInput")
    w1_d = nc.dram_tensor("w1", [128, p.finc, fmid], dt.bfloat16, kind="ExternalInput")
    w2_d = nc.dram_tensor("w2", [fmid, fmid], dt.bfloat16, kind="ExternalInput")
    w3_d = nc.dram_tensor("w3", [fmid, FOUT_PAD], dt.bfloat16, kind="ExternalInput")
    d1_d = nc.dram_tensor("d1", [128, nranks], dt.float32, kind="ExternalInput")
    d2_d = nc.dram_tensor("d2", [128, nranks], dt.float32, kind="ExternalInput")
    d3_d = nc.dram_tensor("d3", [128, nranks], dt.float32, kind="ExternalInput")
    id_d = nc.dram_tensor("ident", [128, 128], dt.bfloat16, kind="ExternalInput")
    il_d = nc.dram_tensor("idx_lo", [128, llo // 16], dt.int16, kind="ExternalInput")
    ih_d = nc.dram_tensor("idx_hi", [128, lhi // 16], dt.int16, kind="ExternalInput")
    ip_d = nc.dram_tensor("idx_pm", [128, sec // 16], dt.int16, kind="ExternalInput")
    out_d = nc.dram_tensor("out", [128, nranks * fout], dt.float32, kind="ExternalOutput")

    ag_in = nc.dram_tensor("ag_in", [128, sec], dt.bfloat16)
    ago = nc.dram_tensor("ago", [C * 128, sec], dt.bfloat16, addr_space="Shared")
    # flat [token, 128] views for the DRAM-source gathers
    ago_lo = ago[0:C // 2 * 128, :].rearrange("a (r f) -> (a r) f", f=128)
    ago_hi = ago[C // 2 * 128:C * 128, :].rearrange("a (r f) -> (a r) f", f=128)

    with tile.TileContext(nc) as tc:
        with (
            tc.tile_pool(name="main", bufs=1) as main,
            tc.tile_pool(name="mp", bufs=4) as mp,
            tc.tile_pool(name="gp", bufs=3) as gp,
            tc.tile_pool(name="psb", bufs=4, space=bass.MemorySpace.PSUM) as psb,
            tc.tile_pool(name="pss", bufs=2, space=bass.MemorySpace.PSUM) as pss,
        ):
            TS = main.tile([128, nranks, 128], dt.bfloat16)
            sA = main.tile([128, 1, sec], dt.bfloat16)
            work = main.tile([128, 1, sec], dt.bfloat16)
            SL = main.tile([128, 1, sec], dt.float32)
            ident32 = main.tile([128, 128], dt.float32)
            pw = main.tile([128, 1, sec], dt.bfloat16)
            ident = main.tile([128, 128], dt.bfloat16)
            w1 = main.tile([128, p.finc, fmid], dt.bfloat16)
            w2 = main.tile([fmid, fmid], dt.bfloat16)
            w3 = main.tile([fmid, FOUT_PAD], dt.bfloat16)
            d1 = main.tile([128, nranks], dt.float32)
            d2 = main.tile([128, nranks], dt.float32)
            d3 = main.tile([128, nranks], dt.float32)
            il = main.tile([128, llo // 16], dt.int16)
            ih = main.tile([128, lhi // 16], dt.int16)
            ip = main.tile([128, sec // 16], dt.int16)

            nc.sync.dma_start(ident[:], id_d[:])
            nc.sync.dma_start(w1[:], w1_d[:])
            nc.sync.dma_start(w2[:], w2_d[:])
            nc.sync.dma_start(w3[:], w3_d[:])
            nc.sync.dma_start(d1[:], d1_d[:])
            nc.sync.dma_start(d2[:], d2_d[:])
            nc.sync.dma_start(d3[:], d3_d[:])
            nc.sync.dma_start(il[:], il_d[:])
            nc.sync.dma_start(ih[:], ih_d[:])
            nc.sync.dma_start(ip[:], ip_d[:])
            nc.vector.tensor_copy(ident32[:], ident[:])
            nc.vector.memset(TS[:], 0.0)
            nc.vector.memset(work[:], 0.0)
            nc.vector.memset(SL[:], 0.0)

            def pstage_l1():
                GRP = 13
                ngrp = (nranks + GRP - 1) // GRP
                xts = []
                for g in range(ngrp):
                    a, b = g * GRP, min(nranks, (g + 1) * GRP)
                    xt = mp.tile([128, b - a, p.finc, 128], dt.bfloat16, tag="m")
                    nc.sync.dma_start(xt[:], xT_d[:, a:b, :, :])
                    xts.append((a, xt))
                for c in range(nranks):
                    g = c // GRP
                    a, xt = xts[g]
                    ps = pss.tile([128, fmid], dt.float32, tag="pp")
                    for f in range(p.finc):
                        nc.tensor.matmul(ps[:], xt[:, c - a, f, :], w1[:, f, :],
                                         start=(f == 0), stop=(f == p.finc - 1))
                    nc.vector.tensor_scalar_mul(work[:, 0, c * 128:(c + 1) * 128],
                                                ps[:], d1[:, c:c + 1])
                    nc.vector.tensor_scalar_mul(SL[:, 0, c * 128:(c + 1) * 128],
                                                ps[:], d2[:, c:c + 1])

            def allgather():
                if sim_mode:
                    # causal stand-in for the collective: a small ago write
                    # ordered after the dense stage's last chunk
                    nc.sync.dma_start(ago[0:128, 0:128],
                                      work[:, 0, sec - 128:sec])
                else:
                    nc.gpsimd.collective_compute(
                        "AllGather", mybir.AluOpType.bypass,
                        replica_groups=[list(range(C))],
                        ins=[ag_in.ap().opt()], outs=[ago.ap().opt()])

            def gather_pass(half, idxt):
                """Issue the DRAM-source gathers for one pass; returns chunk
                tiles + locations."""
                chunks = p.pass_chunks[half]
                cell_loc = p.pass_cell_loc[half]
                in_ap = ago_lo if half == 0 else ago_hi
                coff = [0]
                for l in chunks:
                    coff.append(coff[-1] + l)
                mts = {}
                for ci, clen in enumerate(chunks):
                    m = gp.tile([128, 1, clen], dt.bfloat16, tag="g")
                    nc.gpsimd.dma_gather(
                        out_ap=m[:], in_ap=in_ap,
                        idxs_ap=idxt[:, coff[ci] // 16:(coff[ci] + clen) // 16],
                        num_idxs=clen, num_idxs_reg=clen, elem_size=128,
                        transpose=True, single_packet=False, queue_num=0)
                    mts[ci] = m
                return mts, cell_loc

            def reduction_hi(mts, cell_loc):
                """Hi pass slot matmuls, drain via transpose to TS
                (token-major, hi order)."""
                kmax = {}
                for (b, k) in cell_loc:
                    kmax[b] = max(kmax.get(b, -1), k)
                for b in range(nblk):
                    bsz = min(BLK, npc - b * BLK)
                    ps = psb.tile([128, BLK], dt.float32, tag="ps")
                    for k in range(kmax[b] + 1):
                        ci, off, n = cell_loc[(b, k)]
                        nc.tensor.matmul(ps[:, 0:n], ident[:], mts[ci][:, 0, off:off + n],
                                         start=(k == 0), stop=(k == kmax[b]))
                    sb = mp.tile([128, BLK], dt.bfloat16, tag="sb")
                    nc.vector.tensor_copy(sb[:, 0:bsz], ps[:, 0:bsz])
                    for q in range((bsz + 127) // 128):
                        w_ = min(128, bsz - q * 128)
                        pt = pss.tile([128, 128], dt.bfloat16, tag="pt")
                        nc.tensor.transpose(pt[0:w_, :], sb[:, q * 128:q * 128 + w_],
                                            ident[:])
                        r = (b * BLK) // 128 + q
                        nc.vector.tensor_copy(TS[0:w_, r, :], pt[0:w_, :])

            def perm_gather():
                nc.gpsimd.dma_gather(
                    out_ap=pw[:], in_ap=TS[:], idxs_ap=ip[:],
                    num_idxs=sec, num_idxs_reg=sec, elem_size=128,
                    transpose=True, sbuf_tokens_per_rank=128,
                    sbuf_free_dim_per_rank=256, sbuf_free_dim_pad_per_rank=0,
                    sbuf_byte_offset=0, single_packet=False, queue_num=0)

            def reduction_lo_slot(mts, cell_loc, relu):
                """Lo pass, layers 0/1: SL seed + slot matmuls + permuted-hi
                merge slot in one PSUM accumulation; ACT drain (relu) -> work."""
                kmax = {}
                for (b, k) in cell_loc:
                    kmax[b] = max(kmax.get(b, -1), k)
                for b in range(nblk):
                    a0 = b * BLK
                    bsz = min(BLK, npc - a0)
                    ps = psb.tile([128, BLK], dt.float32, tag="ps")
                    nq = (bsz + 127) // 128
                    for q in range(nq):
                        nc.tensor.matmul(
                            ps[:, q * 128:(q + 1) * 128],
                            SL[:, 0, a0 + q * 128:a0 + (q + 1) * 128],
                            ident32[:], is_transpose=True, start=True, stop=False)
                    for k in range(kmax.get(b, -1) + 1):
                        ci, off, n = cell_loc[(b, k)]
                        nc.tensor.matmul(ps[:, 0:n], ident[:], mts[ci][:, 0, off:off + n],
                                         start=False, stop=False)
                    nc.tensor.matmul(ps[:, 0:bsz], ident[:], pw[:, 0, a0:a0 + bsz],
                                     start=False, stop=True)
                    fn = (mybir.ActivationFunctionType.Relu if relu
                          else mybir.ActivationFunctionType.Copy)
                    nc.scalar.activation(work[:, 0, a0:a0 + bsz], ps[:, 0:bsz], fn)

            def reduction_lo_split(mts, cell_loc):
                """Lo pass, final layer: drain each block to sA with no pw
                dependency so PSUM recycles while gathers stream."""
                kmax = {}
                for (b, k) in cell_loc:
                    kmax[b] = max(kmax.get(b, -1), k)
                for b in range(nblk):
                    a0 = b * BLK
                    bsz = min(BLK, npc - a0)
                    ps = psb.tile([128, BLK], dt.float32, tag="ps")
                    nslot = kmax.get(b, -1) + 1
                    nq = (bsz + 127) // 128
                    for q in range(nq):
                        nc.tensor.matmul(
                            ps[:, q * 128:(q + 1) * 128],
                            SL[:, 0, a0 + q * 128:a0 + (q + 1) * 128],
                            ident32[:], is_transpose=True, start=True,
                            stop=(nslot == 0 and q == nq - 1))
                    for k in range(nslot):
                        ci, off, n = cell_loc[(b, k)]
                        nc.tensor.matmul(ps[:, 0:n], ident[:], mts[ci][:, 0, off:off + n],
                                         start=False, stop=(k == nslot - 1))
                    nc.vector.tensor_copy(sA[:, 0, a0:a0 + bsz], ps[:, 0:bsz])

            def pstage(w):
                for c in range(nranks):
                    ps = pss.tile([128, w.shape[-1]], dt.float32, tag="pp")
                    nc.tensor.matmul(ps[:], work[:, 0, c * 128:(c + 1) * 128], w[:],
                                     start=True, stop=True)
                    psv = ps[:, 0:128] if w.shape[-1] >= 128 else ps[:]
                    nc.vector.tensor_scalar_mul(
                        work[:, 0, c * 128:(c + 1) * 128], psv, d2[:, c:c + 1])
                    nc.vector.tensor_scalar_mul(
                        SL[:, 0, c * 128:(c + 1) * 128], psv, d3[:, c:c + 1])

            def merge_final(sm, et, lg):
                """Final layer, per block: merge sA+pw, transpose to
                token-major, log_softmax, and stream the output DMA."""
                for b in range(nblk):
                    a0 = b * BLK
                    bsz = min(BLK, npc - a0)
                    nc.vector.tensor_tensor(work[:, 0, a0:a0 + bsz],
                                            sA[:, 0, a0:a0 + bsz],
                                            pw[:, 0, a0:a0 + bsz],
                                            mybir.AluOpType.add)
                    cr0, cr1 = 4 * b, min(4 * b + 4, nranks)
                    for c in range(cr0, cr1):
                        pt = pss.tile([128, fout], dt.bfloat16, tag="pt")
                        nc.tensor.matmul(pt[:], work[:, 0, c * 128:(c + 1) * 128],
                                         ident[:, 0:fout], is_transpose=True,
                                         start=True, stop=True)
                        nc.vector.tensor_scalar_mul(sm[:, c, :], pt[:], d1[:, c:c + 1])
                        nc.scalar.activation(et[:, c, :], sm[:, c, :],
                                             mybir.ActivationFunctionType.Exp)
                    nc.vector.reduce_sum(lg[:, cr0:cr1], et[:, cr0:cr1, :],
                                         axis=mybir.AxisListType.X)
                    nc.scalar.activation(lg[:, cr0:cr1], lg[:, cr0:cr1],
                                         mybir.ActivationFunctionType.Ln)
                    for c in range(cr0, cr1):
                        nc.vector.tensor_scalar_sub(sm[:, c, :], sm[:, c, :],
                                                    lg[:, c:c + 1])
                    nc.sync.dma_start(
                        out_d[:, cr0 * fout:cr1 * fout],
                        sm[:, cr0:cr1, :].rearrange("q c f -> q (c f)"))

            # ---- program ----
            pstage_l1()
            nc.sync.dma_start(ag_in[:, :], work[:, 0, :])
            sm = mp.tile([128, nranks, fout], dt.float32, tag="o")
            et = mp.tile([128, nranks, fout], dt.float32, tag="o")
            lg = mp.tile([128, nranks], dt.float32, tag="o")
            for layer in range(3):
                allgather()
                mts_hi, cl_hi = gather_pass(1, ih)
                reduction_hi(mts_hi, cl_hi)
                perm_gather()
                mts_lo, cl_lo = gather_pass(0, il)
                if layer < 2:
                    reduction_lo_slot(mts_lo, cl_lo, relu=True)
                    pstage(w2 if layer == 0 else w3)
                    nc.sync.dma_start(ag_in[:, :], work[:, 0, :])
                else:
                    reduction_lo_split(mts_lo, cl_lo)
                    merge_final(sm, et, lg)
    nc.compile()
    return nc


# --------------------------------------------------------------------------
# Entry point
# --------------------------------------------------------------------------
def _make_in_maps(p, inputs, fmid, fout):
    W_in = np.asarray(inputs["W_in"], dtype=np.float32)
    W_mid = np.asarray(inputs["W_mid"], dtype=np.float32)
    W_out = np.asarray(inputs["W_out"], dtype=np.float32)
    w1 = np.ascontiguousarray(
        W_in.reshape(p.finc, 128, fmid).transpose(1, 0, 2).astype(BF16))
    w2 = np.ascontiguousarray(W_mid.astype(BF16))
    w3 = np.zeros((fmid, FOUT_PAD), BF16)
    w3[:, :fout] = W_out.astype(BF16)
    ident = np.eye(128, dtype=np.float32).astype(BF16)
    in_maps = []
    for m in range(C):
        in_maps.append({
            "xT": p.xT[m].reshape(128, p.nranks, p.finc, 128),
            "w1": w1, "w2": w2, "w3": w3,
            "d1": p.d1[m], "d2": p.d2[m], "d3": p.d3[m], "ident": ident,
            "idx_lo": p.idx_lo[m], "idx_hi": p.idx_hi[m], "idx_pm": p.idx_pm[m],
        })
    return in_maps


def _run(inputs, trace=False, trace_cores=None):
    x = np.asarray(inputs["x"], dtype=np.float32)
    edge_index = np.asarray(inputs["edge_index"])
    W_in = np.asarray(inputs["W_in"], dtype=np.float32)
    W_out = np.asarray(inputs["W_out"], dtype=np.float32)
    for bname in ("b_in", "b_mid", "b_out"):
        if np.any(np.asarray(inputs[bname])):
            raise NotImplementedError("nonzero bias path not implemented")

    N, fin = x.shape
    fmid = W_in.shape[1]
    fout = W_out.shape[1]
    p = preprocess(x, edge_index, N)

    nc = build_kernel(p, fmid, fout)

    in_maps = _make_in_maps(p, inputs, fmid, fout)
    kw = {}
    if trace:
        kw = dict(trace=True, trace_cores=trace_cores or [0])
    r = run_bass_kernel_spmd(nc, in_maps, core_ids=list(range(C)), **kw)

    out = np.empty((N, fout), np.float32)
    for m in range(C):
        res = r.results[m]["out"]  # [128, nranks*fout] partition-major
        rows = res.reshape(128, p.nranks, fout).transpose(1, 0, 2).reshape(p.sec, fout)
        out[m * p.npc + p.perm_lo[m]] = rows[:p.npc]
    return out, r


def kernel(**inputs) -> np.ndarray:
    out, _ = _run(inputs)
    return out
